# revision 1
# baseline (speedup 1.0000x reference)
"""LorentzGNN (2x Lorentz-GAT + readout) Trainium2 kernel, 8 NeuronCores.

Strategy (graph/data parallel, hardcoded from the sharding hint):
  - Core c owns dst nodes [4096c, 4096(c+1)) = 8 whole graphs of 512 nodes.
  - Within a shard, nodes are renumbered by degree (descending) so each
    128-node tile has a uniform padded-CSR depth D_t (max degree in tile).
  - Per layer: sharded node phase computes a 192-f32 record per node
    [z(0:128) | s_src(128) | pad], written to a DRAM table shard;
    AllGather makes the full [32768,192] table visible to every core.
  - Edge phase: per tile, dma_gather pulls the src-records of all incident
    edges into [128 dst-partitions, k-slots, 192]; attention softmax and
    the weighted sum run on DVE/ACT with grouped reduces (no indicator
    matmuls needed because each dst owns one partition).
  - expmap0/projx/logmap0 between layers cancels analytically, so layer-2
    tangent input is just gelu(agg1).
  - Readout (centroid + g-rows + LorentzLinear) is computed on-device per
    core for its 8 graphs; host concatenates the [8,129] shards.
"""
import os
import sys
import copy
import time

sys.path.insert(0, "/opt/trn_rl_repo")

import numpy as np

import concourse.bacc as bacc
import concourse.tile as tile
import concourse.bass as bass
from concourse import mybir, masks
from concourse.bass_utils import run_bass_kernel_spmd

FP = mybir.dt.float32
AF = mybir.ActivationFunctionType
ALU = mybir.AluOpType

N_NODES = 32768
N_EDGES = 524288
FT_IN = 256
HID = 128
BATCH = 64
N_CORES = 8
SHARD = N_NODES // N_CORES      # 4096
TILES = SHARD // 128            # 32
BF16_REC = os.environ.get("K_BF16", "1") == "1"
# record: [z(0:HID) | s_src(HID) | pad]; 768B in f32, 512B in bf16
REC = 256 if BF16_REC else 192
RECD = mybir.dt.bfloat16 if BF16_REC else mybir.dt.float32
KCH = 24                        # max k-slots per gather piece
EPS = 1e-7


# ---------------------------------------------------------------------------
# walrus in this container supports only ONE sync-wait per instruction;
# split extras onto standalone EventSemaphore instructions (same engine,
# immediately before -> program order preserves semantics).
def _split_waits(nc, max_waits=1):
    f = nc.m.functions[0]
    template = None
    for blk in f.blocks:
        for ins in blk.instructions:
            if type(ins).__name__ == "InstEventSemaphore":
                template = ins
                break
        if template is not None:
            break
    assert template is not None
    uid = 0
    for blk in f.blocks:
        new_list = []
        changed = False
        for ins in blk.instructions:
            si = ins.sync_info
            waits = list(si.on_wait) if si is not None else []
            if len(waits) > max_waits:
                keep = waits[-max_waits:]
                for w in waits[: len(waits) - max_waits]:
                    ev = copy.deepcopy(template)
                    ev.name = f"bass_split_wait_{uid}"
                    uid += 1
                    ev.engine = ins.engine
                    nsi = copy.deepcopy(si)
                    nsi.on_wait = [w]
                    nsi.on_update = []
                    ev.sync_info = nsi
                    new_list.append(ev)
                nsi2 = copy.deepcopy(si)
                nsi2.on_wait = keep
                ins.sync_info = nsi2
                changed = True
            new_list.append(ins)
        if changed:
            blk.instructions = new_list


# ---------------------------------------------------------------------------
# Host-side graph preprocessing: sharding, degree-sort renumbering,
# padded-CSR gather indices, masks, per-tile readout indicators.
def _preprocess(edge_index):
    dst = np.asarray(edge_index[0], np.int64)
    src = np.asarray(edge_index[1], np.int64)

    perms = []       # per core: local row j -> original local node
    invperms = []    # per core: original local node -> local row
    degs = []
    for c in range(N_CORES):
        sel = (dst >= SHARD * c) & (dst < SHARD * (c + 1))
        dloc = dst[sel] - SHARD * c
        deg = np.bincount(dloc, minlength=SHARD)
        order = np.argsort(-deg, kind="stable")
        inv = np.empty(SHARD, np.int64)
        inv[order] = np.arange(SHARD)
        perms.append(order)
        invperms.append(inv)
        degs.append(deg)

    # renumbered global row of original node s
    renum = np.empty(N_NODES, np.int64)
    for c in range(N_CORES):
        renum[SHARD * c: SHARD * (c + 1)] = SHARD * c + invperms[c]

    # uniform tile depths across cores
    Dt = np.zeros(TILES, np.int64)
    for c in range(N_CORES):
        sd = degs[c][perms[c]]                      # sorted degrees
        for t in range(TILES):
            Dt[t] = max(Dt[t], sd[128 * t: 128 * (t + 1)].max())
    Dt = np.maximum(Dt, 1)

    # pieces: (tile, k0, kk, first, last, idx_off, mask_off)
    pieces = []
    ioff = moff = 0
    for t in range(TILES):
        k0 = 0
        while k0 < Dt[t]:
            kk = int(min(KCH, Dt[t] - k0))
            pieces.append((t, k0, kk, k0 == 0, k0 + kk == Dt[t], ioff, moff))
            ioff += 8 * kk
            moff += kk
            k0 += kk
    CI, CM = ioff, moff

    # per-core CSR in renumbered order + idx/mask/indicator buffers
    per_core = []
    for c in range(N_CORES):
        sel = (dst >= SHARD * c) & (dst < SHARD * (c + 1))
        dloc = dst[sel] - SHARD * c
        sglob = src[sel]
        eorder = np.argsort(invperms[c][dloc], kind="stable")
        s_sorted = renum[sglob[eorder]]             # src in renumbered space
        deg_r = degs[c][perms[c]]                   # degree per local row
        starts = np.zeros(SHARD + 1, np.int64)
        starts[1:] = np.cumsum(deg_r)

        idx_buf = np.zeros((128, CI), np.int16)
        mask_buf = np.zeros((128, CM), np.float32)
        for (t, k0, kk, _f, _l, io, mo) in pieces:
            lin = np.zeros(128 * kk, np.int64)
            msk = np.zeros((128, kk), np.float32)
            rows = 128 * t + np.arange(128)
            for j in range(128):
                r = rows[j]
                d = deg_r[r]
                lo, hi = k0, min(d, k0 + kk)
                if hi > lo:
                    e0 = starts[r] + lo
                    kks = np.arange(lo, hi) - k0
                    lin[kks * 128 + j] = s_sorted[e0: e0 + (hi - lo)]
                    msk[j, : hi - lo] = 1.0
            wrapped = lin.astype(np.int16).reshape(-1, 16).T   # [16, 8*kk]
            for g in range(8):
                idx_buf[16 * g: 16 * (g + 1), io: io + 8 * kk] = wrapped
            mask_buf[:, mo: mo + kk] = msk

        ind_buf = np.zeros((128, 16 * TILES), np.float32)
        for t in range(TILES):
            for j in range(128):
                orig = SHARD * c + perms[c][128 * t + j]
                gcol = orig // 512 - 8 * c
                ind_buf[j, 16 * t + gcol] = 1.0
                if orig % 512 == 0:
                    ind_buf[j, 16 * t + 8 + gcol] = 1.0

        per_core.append(dict(idx=idx_buf, mask=mask_buf, ind=ind_buf,
                             perm=perms[c]))

    return pieces, CI, CM, per_core


# ---------------------------------------------------------------------------
def _build(pieces, CI, CM):
    n_dev = 1 if os.environ.get("K_SINGLE") else N_CORES
    nc = bacc.Bacc("TRN2", target_bir_lowering=False, debug=False,
                   num_devices=n_dev)
    I = {}
    I["x_perm"] = nc.dram_tensor("x_perm", [SHARD, FT_IN + 1], FP,
                                 kind="ExternalInput")
    I["W1"] = nc.dram_tensor("W1", [FT_IN, HID], FP, kind="ExternalInput")
    I["W2"] = nc.dram_tensor("W2", [HID, HID], FP, kind="ExternalInput")
    I["b1"] = nc.dram_tensor("b1", [1, HID], FP, kind="ExternalInput")
    I["b2"] = nc.dram_tensor("b2", [1, HID], FP, kind="ExternalInput")
    for nm in ("a1s", "a1d", "a2s", "a2d"):
        I[nm] = nc.dram_tensor(nm, [1, HID], FP, kind="ExternalInput")
    I["W_lin"] = nc.dram_tensor("W_lin", [HID + 1, HID + 1], FP,
                                kind="ExternalInput")
    I["lin_scale"] = nc.dram_tensor("lin_scale", [1, 1], FP,
                                    kind="ExternalInput")
    I["idx"] = nc.dram_tensor("idx", [128, CI], mybir.dt.int16,
                              kind="ExternalInput")
    I["mask"] = nc.dram_tensor("mask", [128, CM], FP, kind="ExternalInput")
    I["ind"] = nc.dram_tensor("ind", [128, 16 * TILES], FP,
                              kind="ExternalInput")
    out_sh = nc.dram_tensor("out_shard", [8, HID + 1], FP,
                            kind="ExternalOutput")
    gm_sh = nc.dram_tensor("gm_shard", [8, HID + 1], FP,
                           kind="ExternalOutput")

    REP = int(os.environ.get("K_REPEAT", "1"))
    with tile.TileContext(nc) as tc:
        for _ in range(REP):
            _trace(nc, tc, I, out_sh, gm_sh, pieces)
    nc.compile()
    _split_waits(nc)
    return nc


def _trace(nc, tc, I, out_sh, gm_sh, pieces):
    with (
        tc.tile_pool(name="const", bufs=1) as cpool,
        tc.tile_pool(name="io", bufs=2) as iopool,
        tc.tile_pool(name="gat", bufs=3) as gpool,
        tc.tile_pool(name="vv", bufs=2) as vpool,
        tc.tile_pool(name="sm", bufs=4) as spool,
        tc.tile_pool(name="acc", bufs=2) as apool,
        tc.tile_pool(name="ps", bufs=2, space="PSUM") as ppool,
        tc.tile_pool(name="ps1", bufs=1, space="PSUM") as ppool1,
        tc.tile_pool(name="psg", bufs=1, space="PSUM") as gmpool,
        tc.tile_pool(name="dram", bufs=1, space="DRAM") as dpool,
    ):
        # ---- constants
        ident = cpool.tile([128, 128], FP)
        masks.make_identity(nc, ident[:])
        ones_row = cpool.tile([1, 128], FP)
        nc.vector.memset(ones_row[:], 1.0)

        idx_all = cpool.tile([128, max(I["idx"].shape[1], 16)],
                             mybir.dt.int16)
        nc.sync.dma_start(idx_all[:, 0:I["idx"].shape[1]], I["idx"].ap())
        mask_all = cpool.tile([128, max(I["mask"].shape[1], 4)], FP)
        nc.sync.dma_start(mask_all[:, 0:I["mask"].shape[1]], I["mask"].ap())
        ind_all = cpool.tile([128, 16 * TILES], FP)
        nc.sync.dma_start(ind_all[:], I["ind"].ap())

        W1sb = cpool.tile([128, 2, HID], FP)     # [feat_half, 2, hid]
        nc.sync.dma_start(W1sb[:, 0, :], I["W1"].ap()[0:128, :])
        nc.sync.dma_start(W1sb[:, 1, :], I["W1"].ap()[128:256, :])
        W2sb = cpool.tile([128, HID], FP)
        nc.sync.dma_start(W2sb[:], I["W2"].ap())
        b1r = cpool.tile([1, HID], FP)
        nc.sync.dma_start(b1r[:], I["b1"].ap())
        b2r = cpool.tile([1, HID], FP)
        nc.sync.dma_start(b2r[:], I["b2"].ap())
        Wlin = cpool.tile([128, HID + 1], FP)
        nc.sync.dma_start(Wlin[:], I["W_lin"].ap()[0:128, :])
        Wlin_l = cpool.tile([1, HID + 1], FP)
        nc.sync.dma_start(Wlin_l[:], I["W_lin"].ap()[128:129, :])
        lsc = cpool.tile([1, 1], FP)
        nc.sync.dma_start(lsc[:], I["lin_scale"].ap())

        # replicated a-vectors via PE outer product with ones
        amats = {}
        for nm in ("a1s", "a1d", "a2s", "a2d"):
            row = cpool.tile([1, HID], FP, tag=f"row_{nm}")
            nc.sync.dma_start(row[:], I[nm].ap())
            ps = ppool.tile([128, HID], FP, tag="tr")
            nc.tensor.matmul(ps[:], ones_row[:], row[:], start=True, stop=True)
            m = cpool.tile([128, HID], FP, tag=f"amat_{nm}")
            nc.vector.tensor_copy(m[:], ps[:])
            amats[nm] = m

        # persistent per-layer state
        sd1 = cpool.tile([128, TILES], FP)
        sd2 = cpool.tile([128, TILES], FP)
        agg_all = cpool.tile([128, TILES, HID], FP, tag="agg_all")
        t2_all = cpool.tile([128, TILES, HID], FP, tag="t2_all")
        agg2_all = cpool.tile([128, TILES, HID], FP, tag="agg2_all")
        h2_all = cpool.tile([128, TILES, HID + 1], FP, tag="h2_all")
        n2_all = cpool.tile([128, TILES], FP, tag="n2_all")
        sc32a = cpool.tile([128, TILES], FP, tag="sc32a")
        sc32b = cpool.tile([128, TILES], FP, tag="sc32b")
        sc32c = cpool.tile([128, TILES], FP, tag="sc32c")
        sc32d = cpool.tile([128, TILES], FP, tag="sc32d")
        xall = cpool.tile([128, TILES, FT_IN + 1], FP, tag="xall")

        tab1_sh = dpool.tile([SHARD, REC], RECD)
        tab1 = dpool.tile([N_NODES, REC], RECD)
        tab2_sh = dpool.tile([SHARD, REC], RECD)
        tab2 = dpool.tile([N_NODES, REC], RECD)
        sd1b = cpool.tile([128, TILES], RECD, tag="sd1b")
        sd2b = cpool.tile([128, TILES], RECD, tag="sd2b")

        def node_phase(t, tanT_parts, brow, ams, amd, sd_t, tab_shard):
            """z = tan @ W + b into PSUM; record row + s_src/s_dst."""
            z_ps = ppool.tile([128, HID], FP, tag="z")
            for i, (tT, Wp) in enumerate(tanT_parts):
                nc.tensor.matmul(z_ps[:], tT[:], Wp, start=(i == 0),
                                 stop=False)
            nc.tensor.matmul(z_ps[:], ones_row[:], brow[:], start=False,
                             stop=True)
            stg = iopool.tile([128, REC], RECD, tag="stg")
            nc.scalar.copy(stg[:, 0:HID], z_ps[:])
            scr = vpool.tile([128, HID], FP, tag="scr")
            nc.vector.tensor_tensor(scr[:], z_ps[:], ams[:], ALU.mult)
            ssf = spool.tile([128, 1], FP, tag="ssf")
            nc.vector.tensor_reduce(ssf[:], scr[:],
                                    axis=mybir.AxisListType.X, op=ALU.add)
            nc.vector.tensor_copy(stg[:, HID:HID + 1], ssf[:])
            scr2 = vpool.tile([128, HID], FP, tag="scr2")
            nc.vector.tensor_tensor(scr2[:], z_ps[:], amd[:], ALU.mult)
            nc.vector.tensor_reduce(sd_t[:, t:t + 1], scr2[:],
                                    axis=mybir.AxisListType.X, op=ALU.add)
            nc.sync.dma_start(tab_shard[128 * t:128 * (t + 1), :], stg[:])

        # ============ phase A1: logmap (batched ACT) + conv1 node part ====
        nc.sync.dma_start(
            xall[:], I["x_perm"].ap().rearrange("(t p) f -> p t f", p=128))
        for t in range(TILES):
            scr = vpool.tile([128, FT_IN], FP, tag="scrA")
            nc.vector.tensor_tensor(scr[:], xall[:, t, 1:FT_IN + 1],
                                    xall[:, t, 1:FT_IN + 1], ALU.mult)
            nc.vector.tensor_reduce(n2_all[:, t:t + 1], scr[:],
                                    axis=mybir.AxisListType.X, op=ALU.add)
        nn_a = sc32a
        nc.scalar.sqrt(nn_a[:], n2_all[:])                      # 1 table load
        npx = sc32b
        nc.vector.tensor_tensor(npx[:], nn_a[:], xall[:, :, 0], ALU.add)
        lt = sc32c
        nc.scalar.activation(lt[:], npx[:], AF.Ln)              # 1 table load
        rn = sc32d
        nc.vector.reciprocal(rn[:], nn_a[:])
        cf_a = sc32b                                            # reuse
        nc.vector.tensor_tensor(cf_a[:], lt[:], rn[:], ALU.mult)

        for t in range(TILES):
            tan = iopool.tile([128, FT_IN], FP, tag="tan")
            nc.scalar.mul(tan[:], xall[:, t, 1:FT_IN + 1], cf_a[:, t:t + 1])
            parts = []
            for h in range(2):
                tps = ppool.tile([128, 128], FP, tag="tr")
                nc.tensor.transpose(tps[:], tan[:, 128 * h:128 * (h + 1)],
                                    ident[:])
                tsb = iopool.tile([128, 128], FP, tag=f"tT{h}")
                nc.vector.tensor_copy(tsb[:], tps[:])
                parts.append((tsb, W1sb[:, h, :]))
            node_phase(t, parts, b1r, amats["a1s"], amats["a1d"],
                       sd1, tab1_sh)

        if os.environ.get("K_SINGLE"):
            nc.sync.dma_start(tab1[0:SHARD, :], tab1_sh[:])
        else:
            nc.gpsimd.collective_compute(
                "AllGather", ALU.bypass,
                replica_groups=[list(range(N_CORES))],
                ins=[tab1_sh[:].opt()], outs=[tab1[:].opt()])

        phases = os.environ.get("K_PHASES", "full")
        if phases == "a1":
            z0 = cpool.tile([8, HID + 1], FP, tag="zero")
            nc.vector.memset(z0[:], 0.0)
            nc.sync.dma_start(out_sh.ap(), z0[:])
            nc.sync.dma_start(gm_sh.ap(), z0[:])
            return

        # ============ edge phase (both layers): gather + softmax + agg ====
        def edge_phase(tab, sd_t, sd_b, agg_out):
            nc.vector.tensor_copy(sd_b[:], sd_t[:])
            agg_t = None
            dn_t = None
            for (t, k0, kk, first, last, io, mo) in pieces:
                G = gpool.tile([128, KCH, REC], RECD, tag="G")
                nc.gpsimd.dma_gather(
                    out_ap=G[:, 0:kk, :], in_ap=tab[:, :],
                    idxs_ap=idx_all[:, io:io + 8 * kk],
                    num_idxs=128 * kk, num_idxs_reg=128 * kk, elem_size=REC,
                    single_packet=False)
                w = spool.tile([128, KCH], FP, tag="w")
                nc.vector.tensor_tensor(
                    w[:, 0:kk], G[:, 0:kk, HID],
                    sd_b[:, t:t + 1].broadcast_to([128, kk]), ALU.add)
                nc.vector.scalar_tensor_tensor(
                    w[:, 0:kk], w[:, 0:kk], 0.2, w[:, 0:kk],
                    ALU.mult, ALU.max)
                nc.scalar.activation(w[:, 0:kk], w[:, 0:kk], AF.Exp)
                if first:
                    dn_t = apool.tile([128, 1], FP, tag="dn")
                wm = spool.tile([128, KCH], FP, tag="wm")
                nc.vector.tensor_tensor(wm[:, 0:kk], w[:, 0:kk],
                                        mask_all[:, mo:mo + kk], ALU.mult)
                if first:
                    nc.vector.tensor_reduce(dn_t[:], wm[:, 0:kk],
                                            axis=mybir.AxisListType.X,
                                            op=ALU.add)
                else:
                    dnp = spool.tile([128, 1], FP, tag="dnp")
                    nc.vector.tensor_reduce(dnp[:], wm[:, 0:kk],
                                            axis=mybir.AxisListType.X,
                                            op=ALU.add)
                    nc.vector.tensor_tensor(dn_t[:], dn_t[:], dnp[:], ALU.add)
                if first:
                    agg_t = apool.tile([128, HID], FP, tag="agg")
                # fused multiply-accumulate chain over k-slots
                for k in range(kk):
                    if first and k == 0:
                        nc.vector.tensor_scalar_mul(
                            agg_t[:], G[:, k, 0:HID], wm[:, k:k + 1])
                    else:
                        nc.vector.scalar_tensor_tensor(
                            agg_t[:], G[:, k, 0:HID], wm[:, k:k + 1],
                            agg_t[:], ALU.mult, ALU.add)
                if last:
                    dn2 = spool.tile([128, 1], FP, tag="dn2")
                    nc.vector.tensor_scalar_max(dn2[:], dn_t[:], EPS)
                    rcp = spool.tile([128, 1], FP, tag="rcp")
                    nc.vector.reciprocal(rcp[:], dn2[:])
                    nc.vector.tensor_tensor(
                        agg_out[:, t, :], agg_t[:],
                        rcp[:, 0:1].broadcast_to([128, HID]), ALU.mult)

        # ---- layer 1
        edge_phase(tab1, sd1, sd1b, agg_all)
        # batched gelu (single table load)
        nc.scalar.activation(
            t2_all[:].rearrange("p t f -> p (t f)"),
            agg_all[:].rearrange("p t f -> p (t f)"), AF.Gelu_apprx_tanh)
        for t in range(TILES):
            tps = ppool.tile([128, 128], FP, tag="tr")
            nc.tensor.transpose(tps[:], t2_all[:, t, :], ident[:])
            tsb = iopool.tile([128, 128], FP, tag="t2T")
            nc.vector.tensor_copy(tsb[:], tps[:])
            node_phase(t, [(tsb, W2sb[:])], b2r, amats["a2s"],
                       amats["a2d"], sd2, tab2_sh)

        if os.environ.get("K_SINGLE"):
            nc.sync.dma_start(tab2[0:SHARD, :], tab2_sh[:])
        else:
            nc.gpsimd.collective_compute(
                "AllGather", ALU.bypass,
                replica_groups=[list(range(N_CORES))],
                ins=[tab2_sh[:].opt()], outs=[tab2[:].opt()])

        if phases == "l1":
            z0 = cpool.tile([8, HID + 1], FP, tag="zero")
            nc.vector.memset(z0[:], 0.0)
            nc.sync.dma_start(out_sh.ap(), z0[:])
            nc.sync.dma_start(gm_sh.ap(), z0[:])
            return

        # ---- layer 2
        edge_phase(tab2, sd2, sd2b, agg2_all)
        for t in range(TILES):
            scr = vpool.tile([128, HID], FP, tag="e_scr")
            nc.vector.tensor_tensor(scr[:], agg2_all[:, t, :],
                                    agg2_all[:, t, :], ALU.mult)
            nc.vector.tensor_reduce(n2_all[:, t:t + 1], scr[:],
                                    axis=mybir.AxisListType.X, op=ALU.add)
        # batched expmap scalars: nn, sinh(n)/n
        nn_e = sc32a
        nc.scalar.sqrt(nn_e[:], n2_all[:])                      # table load
        ep = sc32b
        nc.scalar.activation(ep[:], nn_e[:], AF.Exp)            # table load
        em = sc32c
        nc.scalar.activation(em[:], nn_e[:], AF.Exp, scale=-1.0)
        sh = sc32b                                              # reuse ep slot
        nc.vector.tensor_tensor(sh[:], ep[:], em[:], ALU.subtract)
        nm = sc32c
        nc.vector.tensor_scalar_max(nm[:], nn_e[:], EPS)
        rn_e = sc32d
        nc.vector.reciprocal(rn_e[:], nm[:])
        cf_e = sc32b
        nc.vector.tensor_tensor(cf_e[:], sh[:], rn_e[:], ALU.mult)
        nc.vector.tensor_scalar_mul(cf_e[:], cf_e[:], 0.5)
        hn2_all = sc32c
        for t in range(TILES):
            nc.scalar.mul(h2_all[:, t, 1:HID + 1], agg2_all[:, t, :],
                          cf_e[:, t:t + 1])
            scr = vpool.tile([128, HID], FP, tag="e_scr2")
            nc.vector.tensor_tensor(scr[:], h2_all[:, t, 1:HID + 1],
                                    h2_all[:, t, 1:HID + 1], ALU.mult)
            nc.vector.tensor_reduce(hn2_all[:, t:t + 1], scr[:],
                                    axis=mybir.AxisListType.X, op=ALU.add)
        # h0 = sqrt(1 + |hs|^2), strided write into h2_all[:, :, 0]
        nc.scalar.activation(h2_all[:, :, 0], hn2_all[:], AF.Sqrt, bias=1.0)

        gm_ps = gmpool.tile([8, HID + 1], FP, tag="gmA")
        g_ps = gmpool.tile([8, HID + 1], FP, tag="gmB")
        for t in range(TILES):
            nc.tensor.matmul(gm_ps[:], ind_all[:, 16 * t:16 * t + 8],
                             h2_all[:, t, :], start=(t == 0),
                             stop=(t == TILES - 1))
            nc.tensor.matmul(g_ps[:], ind_all[:, 16 * t + 8:16 * (t + 1)],
                             h2_all[:, t, :], start=(t == 0),
                             stop=(t == TILES - 1))

        # ================= readout =================
        g = cpool.tile([8, HID + 1], FP, tag="f_g")
        nc.vector.tensor_copy(g[:], g_ps[:])
        ave = cpool.tile([8, HID + 1], FP)
        nc.scalar.mul(ave[:], gm_ps[:], 1.0 / 512.0)
        q = cpool.tile([8, 1], FP, tag="f_q")
        scr = vpool.tile([8, HID], FP, tag="f_scr")
        nc.vector.tensor_tensor(scr[:], ave[:, 1:HID + 1],
                                ave[:, 1:HID + 1], ALU.mult)
        nc.vector.tensor_reduce(q[:], scr[:],
                                axis=mybir.AxisListType.X, op=ALU.add)
        t0s = cpool.tile([8, 1], FP, tag="f_t0s")
        nc.vector.tensor_tensor(t0s[:], ave[:, 0:1], ave[:, 0:1], ALU.mult)
        dif = cpool.tile([8, 1], FP, tag="f_dif")
        nc.vector.tensor_tensor(dif[:], t0s[:], q[:], ALU.subtract)
        nc.vector.tensor_scalar_max(dif[:], dif[:], 1e-8)
        dsq = cpool.tile([8, 1], FP, tag="f_dsq")
        nc.scalar.sqrt(dsq[:], dif[:])
        rr = cpool.tile([8, 1], FP, tag="f_rr")
        nc.vector.reciprocal(rr[:], dsq[:])
        gm = cpool.tile([8, HID + 1], FP, tag="f_gm")
        nc.scalar.mul(gm[:], ave[:], rr[:, 0:1])
        nc.sync.dma_start(gm_sh.ap(), gm[:])

        # y = g @ W_lin
        gT_ps = ppool.tile([128, 8], FP, tag="tr")
        nc.tensor.transpose(gT_ps[:], g[:, 0:128], ident[0:8, 0:8])
        gT = cpool.tile([128, 8], FP, tag="f_gT")
        nc.vector.tensor_copy(gT[:], gT_ps[:])
        gl_ps = ppool1.tile([1, 8], FP, tag="tr2")
        nc.tensor.transpose(gl_ps[:], g[:, 128:129], ident[0:8, 0:8])
        gl = cpool.tile([1, 8], FP, tag="f_gl")
        nc.vector.tensor_copy(gl[:], gl_ps[:])
        y_ps = ppool1.tile([8, HID + 1], FP, tag="y")
        nc.tensor.matmul(y_ps[:], gT[:], Wlin[:], start=True, stop=False)
        nc.tensor.matmul(y_ps[:], gl[:], Wlin_l[:], start=False, stop=True)
        y = cpool.tile([8, HID + 1], FP, tag="f_y")
        nc.vector.tensor_copy(y[:], y_ps[:])

        ls_ps = ppool1.tile([8, 1], FP, tag="tr2")
        ones8 = cpool.tile([1, 8], FP, tag="f_ones8")
        nc.vector.memset(ones8[:], 1.0)
        nc.tensor.matmul(ls_ps[:], ones8[:], lsc[:], start=True, stop=True)
        lsb = cpool.tile([8, 1], FP, tag="f_lsb")
        nc.vector.tensor_copy(lsb[:], ls_ps[:])

        sig = cpool.tile([8, 1], FP, tag="f_sig")
        nc.scalar.activation(sig[:], y[:, 0:1], AF.Sigmoid)
        tme = cpool.tile([8, 1], FP, tag="f_tme")
        nc.vector.tensor_tensor(tme[:], sig[:], lsb[:], ALU.mult)
        nc.vector.tensor_scalar_add(tme[:], tme[:], 1.1)
        s2 = cpool.tile([8, 1], FP, tag="f_s2")
        scr2 = vpool.tile([8, HID], FP, tag="f_scr2")
        nc.vector.tensor_tensor(scr2[:], y[:, 1:HID + 1],
                                y[:, 1:HID + 1], ALU.mult)
        nc.vector.tensor_reduce(s2[:], scr2[:],
                                axis=mybir.AxisListType.X, op=ALU.add)
        nc.vector.tensor_scalar_max(s2[:], s2[:], 1e-8)
        rs2 = cpool.tile([8, 1], FP, tag="f_rs2")
        nc.vector.reciprocal(rs2[:], s2[:])
        tm1 = cpool.tile([8, 1], FP, tag="f_tm1")
        nc.vector.scalar_tensor_tensor(tm1[:], tme[:], 1.0, tme[:],
                                       ALU.mult, ALU.mult)
        nc.vector.tensor_scalar_add(tm1[:], tm1[:], -1.0)
        fac2 = cpool.tile([8, 1], FP, tag="f_fac2")
        nc.vector.tensor_tensor(fac2[:], tm1[:], rs2[:], ALU.mult)
        fac = cpool.tile([8, 1], FP, tag="f_fac")
        nc.scalar.sqrt(fac[:], fac2[:])
        outt = cpool.tile([8, HID + 1], FP, tag="f_out")
        nc.vector.tensor_copy(outt[:, 0:1], tme[:])
        nc.scalar.mul(outt[:, 1:HID + 1], y[:, 1:HID + 1], fac[:, 0:1])
        nc.sync.dma_start(out_sh.ap(), outt[:])


_CACHE = {}


def _get_compiled(edge_index):
    key = hash(np.asarray(edge_index).tobytes())
    if key not in _CACHE:
        pieces, CI, CM, per_core = _preprocess(edge_index)
        nc = _build(pieces, CI, CM)
        _CACHE[key] = (nc, per_core)
    return _CACHE[key]


def kernel(x, edge_index, batch_size, W1, b1, a1_src, a1_dst,
           W2, b2, a2_src, a2_dst, W_lin, lin_scale, _trace=False):
    x = np.asarray(x, np.float32)
    assert int(batch_size) == BATCH
    nc, per_core = _get_compiled(edge_index)

    in_maps = []
    for c in range(N_CORES):
        pc = per_core[c]
        xp = x[SHARD * c + pc["perm"], :]
        in_maps.append(dict(
            x_perm=np.ascontiguousarray(xp),
            W1=np.asarray(W1, np.float32),
            W2=np.asarray(W2, np.float32),
            b1=np.asarray(b1, np.float32).reshape(1, HID),
            b2=np.asarray(b2, np.float32).reshape(1, HID),
            a1s=np.asarray(a1_src, np.float32).reshape(1, HID),
            a1d=np.asarray(a1_dst, np.float32).reshape(1, HID),
            a2s=np.asarray(a2_src, np.float32).reshape(1, HID),
            a2d=np.asarray(a2_dst, np.float32).reshape(1, HID),
            W_lin=np.asarray(W_lin, np.float32),
            lin_scale=np.asarray(lin_scale, np.float32).reshape(1, 1),
            idx=pc["idx"], mask=pc["mask"], ind=pc["ind"],
        ))
    res = run_bass_kernel_spmd(nc, in_maps, core_ids=list(range(N_CORES)),
                               trace=_trace)
    out = np.concatenate([res.results[c]["out_shard"]
                          for c in range(N_CORES)], 0)
    gm = np.concatenate([res.results[c]["gm_shard"]
                         for c in range(N_CORES)], 0)
    if _trace:
        kernel.last_exec_time_ns = res.exec_time_ns
        kernel.last_results = res
    return (out, gm)


kernel.last_exec_time_ns = None


def timed_run(x, edge_index, batch_size, W1, b1, a1_src, a1_dst,
              W2, b2, a2_src, a2_dst, W_lin, lin_scale, iters=30):
    """Run the compiled kernel repeatedly with device-resident inputs and
    report the median per-call wall time (dispatch overhead included) and
    the 10th percentile as a lower bound."""
    import jax
    from jax.sharding import Mesh, PartitionSpec
    from jax.experimental.shard_map import shard_map
    from concourse import bass2jax
    from concourse.bass2jax import _bass_exec_p, partition_id_tensor

    x = np.asarray(x, np.float32)
    nc, per_core = _get_compiled(edge_index)

    in_maps = []
    for c in range(N_CORES):
        pc = per_core[c]
        xp = x[SHARD * c + pc["perm"], :]
        in_maps.append(dict(
            x_perm=np.ascontiguousarray(xp),
            W1=np.asarray(W1, np.float32),
            W2=np.asarray(W2, np.float32),
            b1=np.asarray(b1, np.float32).reshape(1, HID),
            b2=np.asarray(b2, np.float32).reshape(1, HID),
            a1s=np.asarray(a1_src, np.float32).reshape(1, HID),
            a1d=np.asarray(a1_dst, np.float32).reshape(1, HID),
            a2s=np.asarray(a2_src, np.float32).reshape(1, HID),
            a2d=np.asarray(a2_dst, np.float32).reshape(1, HID),
            W_lin=np.asarray(W_lin, np.float32),
            lin_scale=np.asarray(lin_scale, np.float32).reshape(1, 1),
            idx=pc["idx"], mask=pc["mask"], ind=pc["ind"],
        ))

    bass2jax.install_neuronx_cc_hook()
    partition_name = (nc.partition_id_tensor.name
                      if nc.partition_id_tensor else None)
    in_names, out_names, out_avals, zero_outs = [], [], [], []
    for alloc in nc.m.functions[0].allocations:
        if not isinstance(alloc, mybir.MemoryLocationSet):
            continue
        name = alloc.memorylocations[0].name
        if alloc.kind == "ExternalInput":
            if name != partition_name:
                in_names.append(name)
        elif alloc.kind == "ExternalOutput":
            shape = tuple(alloc.tensor_shape)
            dtype = mybir.dt.np(alloc.dtype)
            out_names.append(name)
            out_avals.append(jax.core.ShapedArray(shape, dtype))
            zero_outs.append(np.zeros(shape, dtype))
    n_params = len(in_names)
    n_outs = len(out_avals)
    in_names_all = in_names + out_names
    if partition_name is not None:
        in_names_all = in_names_all + [partition_name]
    donate = tuple(range(n_params, n_params + n_outs))

    def _body(*args):
        operands = list(args)
        if partition_name is not None:
            operands.append(partition_id_tensor())
        outs = _bass_exec_p.bind(
            *operands, out_avals=tuple(out_avals),
            in_names=tuple(in_names_all), out_names=tuple(out_names),
            lowering_input_output_aliases=(),
            sim_require_finite=True, sim_require_nnan=True, nc=nc)
        return tuple(outs)

    devices = jax.devices()[:N_CORES]
    mesh = Mesh(np.asarray(devices), ("core",))
    sharded = jax.jit(
        shard_map(_body, mesh=mesh,
                  in_specs=(PartitionSpec("core"),) * (n_params + n_outs),
                  out_specs=(PartitionSpec("core"),) * n_outs,
                  check_rep=False),
        donate_argnums=donate, keep_unused=True)

    concat_in = [np.concatenate([np.asarray(in_maps[c][nm])
                                 for c in range(N_CORES)], axis=0)
                 for nm in in_names]
    dev_in = [jax.device_put(a) for a in concat_in]
    for a in dev_in:
        a.block_until_ready()

    def one_call():
        zz = [np.zeros((N_CORES * z.shape[0], *z.shape[1:]), z.dtype)
              for z in zero_outs]
        outs = sharded(*dev_in, *zz)
        for o in outs:
            o.block_until_ready()
        return outs

    one_call(); one_call()
    ts = []
    for _ in range(iters):
        t0 = time.perf_counter()
        one_call()
        t1 = time.perf_counter()
        ts.append(t1 - t0)
    ts = np.array(ts)
    return dict(median_s=float(np.median(ts)), p10_s=float(np.percentile(ts, 10)),
                min_s=float(ts.min()), all=ts)



# revision 5
# speedup vs baseline: 1.0059x; 1.0059x over previous
"""LorentzGNN (2x Lorentz-GAT + readout) Trainium2 kernel, 8 NeuronCores.

Strategy (graph/data parallel, hardcoded from the sharding hint):
  - Core c owns dst nodes [4096c, 4096(c+1)) = 8 whole graphs of 512 nodes.
  - Within a shard, nodes are renumbered by degree (descending) so each
    128-node tile has a uniform padded-CSR depth D_t (max degree in tile).
  - Per layer: sharded node phase computes a 256-el bf16 record per node
    [z(0:128) | s_src(128) | pad], written to a DRAM table shard;
    AllGather (2 chunks, overlapped with compute) makes the full table
    visible to every core.
  - Edge phase: ONE dma_gather per dst-tile pulls the src-records of all
    incident edges into [128 dst-partitions, D_t slots, 256]; attention
    weights are computed as [128, D_t] ops, applied with a single big
    elementwise multiply, and reduced over slots with one strided-axis
    tensor_reduce (no per-slot MAC chain).
  - expmap0/projx/logmap0 between layers cancels analytically, so layer-2
    tangent input is just gelu(agg1).
  - Readout (centroid + g-rows + LorentzLinear) is computed on-device per
    core for its 8 graphs; host concatenates the [8,129] shards.
"""
import os
import sys
import copy
import time

sys.path.insert(0, "/opt/trn_rl_repo")

import numpy as np

import concourse.bacc as bacc
import concourse.tile as tile
import concourse.bass as bass
from concourse import mybir, masks
from concourse.bass_utils import run_bass_kernel_spmd

FP = mybir.dt.float32
BF = mybir.dt.bfloat16
AF = mybir.ActivationFunctionType
ALU = mybir.AluOpType

N_NODES = 32768
N_EDGES = 524288
FT_IN = 256
HID = 128
BATCH = 64
N_CORES = 8
SHARD = N_NODES // N_CORES      # 4096
TILES = SHARD // 128            # 32
N_CHUNK = 2                     # AllGather chunks per layer
CTILES = TILES // N_CHUNK       # tiles per chunk
REC = 256                       # record: [z(0:128) | s_src(128) | pad], bf16
EPS = 1e-7


# ---------------------------------------------------------------------------
# walrus in this container supports only ONE sync-wait per instruction;
# split extras onto standalone EventSemaphore instructions (same engine,
# immediately before -> program order preserves semantics).
def _split_waits(nc, max_waits=1):
    f = nc.m.functions[0]
    template = None
    for blk in f.blocks:
        for ins in blk.instructions:
            if type(ins).__name__ == "InstEventSemaphore":
                template = ins
                break
        if template is not None:
            break
    assert template is not None
    uid = 0
    for blk in f.blocks:
        new_list = []
        changed = False
        for ins in blk.instructions:
            si = ins.sync_info
            waits = list(si.on_wait) if si is not None else []
            if len(waits) > max_waits:
                keep = waits[-max_waits:]
                for w in waits[: len(waits) - max_waits]:
                    ev = copy.deepcopy(template)
                    ev.name = f"bass_split_wait_{uid}"
                    uid += 1
                    ev.engine = ins.engine
                    nsi = copy.deepcopy(si)
                    nsi.on_wait = [w]
                    nsi.on_update = []
                    ev.sync_info = nsi
                    new_list.append(ev)
                nsi2 = copy.deepcopy(si)
                nsi2.on_wait = keep
                ins.sync_info = nsi2
                changed = True
            new_list.append(ins)
        if changed:
            blk.instructions = new_list


# ---------------------------------------------------------------------------
# Host-side graph preprocessing: sharding, degree-sort renumbering,
# whole-tile padded-CSR gather indices, masks, per-tile readout indicators.
#
# Global table row for (core c, local degree-sorted row l):
#   chunk = l // (SHARD//N_CHUNK); row = chunk*(N_NODES//N_CHUNK)
#           + (SHARD//N_CHUNK)*c + (l % (SHARD//N_CHUNK))
# so an AllGather of chunk j (concat of all cores' chunk-j shard slices)
# lands records exactly at their global rows.
def _preprocess(edge_index):
    dst = np.asarray(edge_index[0], np.int64)
    src = np.asarray(edge_index[1], np.int64)
    CH_SH = SHARD // N_CHUNK          # local rows per chunk
    CH_GL = N_NODES // N_CHUNK        # global rows per chunk

    perms = []       # per core: local row j -> original local node
    invperms = []    # per core: original local node -> local row
    degs = []
    for c in range(N_CORES):
        sel = (dst >= SHARD * c) & (dst < SHARD * (c + 1))
        dloc = dst[sel] - SHARD * c
        deg = np.bincount(dloc, minlength=SHARD)
        order = np.argsort(-deg, kind="stable")
        inv = np.empty(SHARD, np.int64)
        inv[order] = np.arange(SHARD)
        perms.append(order)
        invperms.append(inv)
        degs.append(deg)

    # renumbered global table row of original node s (chunk-major layout)
    renum = np.empty(N_NODES, np.int64)
    for c in range(N_CORES):
        ell = invperms[c]
        renum[SHARD * c: SHARD * (c + 1)] = (
            (ell // CH_SH) * CH_GL + CH_SH * c + (ell % CH_SH))

    # uniform tile depths across cores
    Dt = np.zeros(TILES, np.int64)
    for c in range(N_CORES):
        sd = degs[c][perms[c]]                      # sorted degrees
        for t in range(TILES):
            Dt[t] = max(Dt[t], sd[128 * t: 128 * (t + 1)].max())
    Dt = np.maximum(Dt, 1)

    # one piece per tile: (tile, D_t, idx_off, mask_off)
    pieces = []
    ioff = moff = 0
    for t in range(TILES):
        pieces.append((t, int(Dt[t]), ioff, moff))
        ioff += 8 * int(Dt[t])
        moff += int(Dt[t])
    CI, CM = ioff, moff

    # per-core CSR in renumbered order + idx/mask/indicator buffers
    per_core = []
    for c in range(N_CORES):
        sel = (dst >= SHARD * c) & (dst < SHARD * (c + 1))
        dloc = dst[sel] - SHARD * c
        sglob = src[sel]
        eorder = np.argsort(invperms[c][dloc], kind="stable")
        s_sorted = renum[sglob[eorder]]             # src table rows
        deg_r = degs[c][perms[c]]                   # degree per local row
        starts = np.zeros(SHARD + 1, np.int64)
        starts[1:] = np.cumsum(deg_r)

        idx_buf = np.zeros((128, CI), np.int16)
        mask_buf = np.zeros((128, CM), np.float32)
        for (t, kk, io, mo) in pieces:
            lin = np.zeros(128 * kk, np.int64)
            msk = np.zeros((128, kk), np.float32)
            rows = 128 * t + np.arange(128)
            for j in range(128):
                r = rows[j]
                d = deg_r[r]
                if d > 0:
                    e0 = starts[r]
                    kks = np.arange(d)
                    lin[kks * 128 + j] = s_sorted[e0: e0 + d]
                    msk[j, :d] = 1.0
            wrapped = lin.astype(np.int16).reshape(-1, 16).T   # [16, 8*kk]
            for g in range(8):
                idx_buf[16 * g: 16 * (g + 1), io: io + 8 * kk] = wrapped
            mask_buf[:, mo: mo + kk] = msk

        ind_buf = np.zeros((128, 16 * TILES), np.float32)
        for t in range(TILES):
            for j in range(128):
                orig = SHARD * c + perms[c][128 * t + j]
                gcol = orig // 512 - 8 * c
                ind_buf[j, 16 * t + gcol] = 1.0
                if orig % 512 == 0:
                    ind_buf[j, 16 * t + 8 + gcol] = 1.0

        per_core.append(dict(idx=idx_buf, mask=mask_buf, ind=ind_buf,
                             perm=perms[c]))

    return pieces, CI, CM, per_core


# ---------------------------------------------------------------------------
def _build(pieces, CI, CM):
    n_dev = 1 if os.environ.get("K_SINGLE") else N_CORES
    nc = bacc.Bacc("TRN2", target_bir_lowering=False, debug=False,
                   num_devices=n_dev)
    I = {}
    I["x_perm"] = nc.dram_tensor("x_perm", [SHARD, FT_IN + 1], FP,
                                 kind="ExternalInput")
    I["W1"] = nc.dram_tensor("W1", [FT_IN, HID], FP, kind="ExternalInput")
    I["W2"] = nc.dram_tensor("W2", [HID, HID], FP, kind="ExternalInput")
    I["b1"] = nc.dram_tensor("b1", [1, HID], FP, kind="ExternalInput")
    I["b2"] = nc.dram_tensor("b2", [1, HID], FP, kind="ExternalInput")
    for nm in ("a1s", "a1d", "a2s", "a2d"):
        I[nm] = nc.dram_tensor(nm, [1, HID], FP, kind="ExternalInput")
    I["W_lin"] = nc.dram_tensor("W_lin", [HID + 1, HID + 1], FP,
                                kind="ExternalInput")
    I["lin_scale"] = nc.dram_tensor("lin_scale", [1, 1], FP,
                                    kind="ExternalInput")
    I["idx"] = nc.dram_tensor("idx", [128, CI], mybir.dt.int16,
                              kind="ExternalInput")
    I["mask"] = nc.dram_tensor("mask", [128, CM], FP, kind="ExternalInput")
    I["ind"] = nc.dram_tensor("ind", [128, 16 * TILES], FP,
                              kind="ExternalInput")
    out_sh = nc.dram_tensor("out_shard", [8, HID + 1], FP,
                            kind="ExternalOutput")
    gm_sh = nc.dram_tensor("gm_shard", [8, HID + 1], FP,
                           kind="ExternalOutput")

    REP = int(os.environ.get("K_REPEAT", "1"))
    with tile.TileContext(nc) as tc:
        for _ in range(REP):
            _trace(nc, tc, I, out_sh, gm_sh, pieces)
    nc.compile()
    _split_waits(nc)
    return nc


def _trace(nc, tc, I, out_sh, gm_sh, pieces):
    DMAX = max(kk for (_t, kk, _io, _mo) in pieces)
    with (
        tc.tile_pool(name="const", bufs=1) as cpool,
        tc.tile_pool(name="io", bufs=2) as iopool,
        tc.tile_pool(name="gat", bufs=2) as gpool,
        tc.tile_pool(name="wg", bufs=2) as wgpool,
        tc.tile_pool(name="vv", bufs=2) as vpool,
        tc.tile_pool(name="sm", bufs=4) as spool,
        tc.tile_pool(name="ps", bufs=2, space="PSUM") as ppool,
        tc.tile_pool(name="ps1", bufs=1, space="PSUM") as ppool1,
        tc.tile_pool(name="psg", bufs=1, space="PSUM") as gmpool,
        tc.tile_pool(name="dram", bufs=1, space="DRAM") as dpool,
    ):
        # ---- constants
        ident = cpool.tile([128, 128], BF)
        masks.make_identity(nc, ident[:])
        ident8 = cpool.tile([8, 8], FP)
        masks.make_identity(nc, ident8[:])
        ones_row = cpool.tile([1, 128], FP)
        nc.vector.memset(ones_row[:], 1.0)

        idx_all = cpool.tile([128, max(I["idx"].shape[1], 16)],
                             mybir.dt.int16)
        nc.sync.dma_start(idx_all[:, 0:I["idx"].shape[1]], I["idx"].ap())
        mask_all = cpool.tile([128, max(I["mask"].shape[1], 4)], BF)
        nc.gpsimd.dma_start(mask_all[:, 0:I["mask"].shape[1]],
                            I["mask"].ap())
        ind_all = cpool.tile([128, 16 * TILES], FP)
        nc.sync.dma_start(ind_all[:], I["ind"].ap())

        # weights as bf16 lhsT tiles (cast on ACT after f32 load)
        W1f = iopool.tile([128, 2, HID], FP, tag="w1f")
        nc.sync.dma_start(W1f[:, 0, :], I["W1"].ap()[0:128, :])
        nc.sync.dma_start(W1f[:, 1, :], I["W1"].ap()[128:256, :])
        W1sb = cpool.tile([128, 2, HID], BF)
        nc.vector.tensor_copy(W1sb[:].rearrange("p a h -> p (a h)"),
                              W1f[:].rearrange("p a h -> p (a h)"))
        W2f = iopool.tile([128, HID], FP, tag="w2f")
        nc.sync.dma_start(W2f[:], I["W2"].ap())
        W2sb = cpool.tile([128, HID], BF)
        nc.vector.tensor_copy(W2sb[:], W2f[:])
        b1r = cpool.tile([1, HID], FP)
        nc.sync.dma_start(b1r[:], I["b1"].ap())
        b2r = cpool.tile([1, HID], FP)
        nc.sync.dma_start(b2r[:], I["b2"].ap())
        Wlin = cpool.tile([128, HID + 1], FP)
        nc.sync.dma_start(Wlin[:], I["W_lin"].ap()[0:128, :])
        Wlin_l = cpool.tile([1, HID + 1], FP)
        nc.sync.dma_start(Wlin_l[:], I["W_lin"].ap()[128:129, :])
        lsc = cpool.tile([1, 1], FP)
        nc.sync.dma_start(lsc[:], I["lin_scale"].ap())

        # replicated a-vectors via PE outer product with ones
        amats = {}
        for nm in ("a1s", "a1d", "a2s", "a2d"):
            row = cpool.tile([1, HID], FP, tag=f"row_{nm}")
            nc.sync.dma_start(row[:], I[nm].ap())
            ps = ppool.tile([128, HID], FP, tag="tr")
            nc.tensor.matmul(ps[:], ones_row[:], row[:], start=True, stop=True)
            m = cpool.tile([128, HID], FP, tag=f"amat_{nm}")
            nc.vector.tensor_copy(m[:], ps[:])
            amats[nm] = m

        # persistent per-layer state
        sd1 = cpool.tile([128, TILES], FP)
        sd2 = cpool.tile([128, TILES], FP)
        agg_all = cpool.tile([128, TILES, HID], FP, tag="agg_all")
        t2_all = cpool.tile([128, TILES, HID], BF, tag="t2_all")
        agg2_all = cpool.tile([128, TILES, HID], FP, tag="agg2_all")
        h2_all = cpool.tile([128, TILES, HID + 1], FP, tag="h2_all")
        n2_all = cpool.tile([128, TILES], FP, tag="n2_all")
        sc32a = cpool.tile([128, TILES], FP, tag="sc32a")
        sc32b = cpool.tile([128, TILES], FP, tag="sc32b")
        sc32c = cpool.tile([128, TILES], FP, tag="sc32c")
        sc32d = cpool.tile([128, TILES], FP, tag="sc32d")
        xall = cpool.tile([128, TILES, FT_IN + 1], FP, tag="xall")

        tab1_sh = dpool.tile([SHARD, REC], BF)
        tab1 = dpool.tile([N_NODES, REC], BF)
        tab2_sh = dpool.tile([SHARD, REC], BF)
        tab2 = dpool.tile([N_NODES, REC], BF)

        CH_SH = SHARD // N_CHUNK
        CH_GL = N_NODES // N_CHUNK

        def ag_chunk(tab_sh, tab, j):
            if os.environ.get("K_SINGLE"):
                nc.sync.dma_start(
                    tab[CH_GL * j: CH_GL * j + CH_SH, :],
                    tab_sh[CH_SH * j: CH_SH * (j + 1), :])
            else:
                nc.gpsimd.collective_compute(
                    "AllGather", ALU.bypass,
                    replica_groups=[list(range(N_CORES))],
                    ins=[tab_sh[CH_SH * j: CH_SH * (j + 1), :].opt()],
                    outs=[tab[CH_GL * j: CH_GL * (j + 1), :].opt()])

        def node_phase(t, tanT_parts, brow, ams, amd, sd_t, tab_shard):
            """z = tan @ W + b into PSUM; record row + s_src/s_dst."""
            z_ps = ppool.tile([128, HID], FP, tag="z")
            for i, (tT, Wp) in enumerate(tanT_parts):
                nc.tensor.matmul(z_ps[:], tT[:], Wp, start=(i == 0),
                                 stop=False)
            nc.tensor.matmul(z_ps[:], ones_row[:], brow[:], start=False,
                             stop=True)
            stg = iopool.tile([128, REC], BF, tag="stg")
            nc.scalar.copy(stg[:, 0:HID], z_ps[:])
            scr = vpool.tile([128, HID], FP, tag="scr")
            nc.vector.tensor_tensor(scr[:], z_ps[:], ams[:], ALU.mult)
            ssf = spool.tile([128, 1], FP, tag="ssf")
            nc.vector.tensor_reduce(ssf[:], scr[:],
                                    axis=mybir.AxisListType.X, op=ALU.add)
            nc.vector.tensor_copy(stg[:, HID:HID + 1], ssf[:])
            scr2 = vpool.tile([128, HID], FP, tag="scr2")
            nc.vector.tensor_tensor(scr2[:], z_ps[:], amd[:], ALU.mult)
            nc.vector.tensor_reduce(sd_t[:, t:t + 1], scr2[:],
                                    axis=mybir.AxisListType.X, op=ALU.add)
            nc.sync.dma_start(tab_shard[128 * t:128 * (t + 1), :], stg[:])

        # ============ phase A1: logmap (batched ACT) + conv1 node part ====
        nc.sync.dma_start(
            xall[:], I["x_perm"].ap().rearrange("(t p) f -> p t f", p=128))
        for t in range(TILES):
            scr = vpool.tile([128, FT_IN], FP, tag="scrA")
            nc.vector.tensor_tensor(scr[:], xall[:, t, 1:FT_IN + 1],
                                    xall[:, t, 1:FT_IN + 1], ALU.mult)
            nc.vector.tensor_reduce(n2_all[:, t:t + 1], scr[:],
                                    axis=mybir.AxisListType.X, op=ALU.add)
        nn_a = sc32a
        nc.scalar.sqrt(nn_a[:], n2_all[:])                      # 1 table load
        npx = sc32b
        nc.vector.tensor_tensor(npx[:], nn_a[:], xall[:, :, 0], ALU.add)
        lt = sc32c
        nc.scalar.activation(lt[:], npx[:], AF.Ln)              # 1 table load
        rn = sc32d
        nc.vector.reciprocal(rn[:], nn_a[:])
        cf_a = sc32b                                            # reuse
        nc.vector.tensor_tensor(cf_a[:], lt[:], rn[:], ALU.mult)

        for j in range(N_CHUNK):
            for t in range(CTILES * j, CTILES * (j + 1)):
                tan = iopool.tile([128, FT_IN], BF, tag="tan")
                nc.scalar.mul(tan[:], xall[:, t, 1:FT_IN + 1],
                              cf_a[:, t:t + 1])
                parts = []
                for h in range(2):
                    tps = ppool.tile([128, 128], BF, tag="tr")
                    nc.tensor.transpose(tps[:],
                                        tan[:, 128 * h:128 * (h + 1)],
                                        ident[:])
                    tsb = iopool.tile([128, 128], BF, tag=f"tT{h}")
                    nc.vector.tensor_copy(tsb[:], tps[:])
                    parts.append((tsb, W1sb[:, h, :]))
                node_phase(t, parts, b1r, amats["a1s"], amats["a1d"],
                           sd1, tab1_sh)
            ag_chunk(tab1_sh, tab1, j)

        phases = os.environ.get("K_PHASES", "full")
        if phases == "a1":
            z0 = cpool.tile([8, HID + 1], FP, tag="zero")
            nc.vector.memset(z0[:], 0.0)
            nc.sync.dma_start(out_sh.ap(), z0[:])
            nc.sync.dma_start(gm_sh.ap(), z0[:])
            return

        # ============ edge phase (one tile = one gather + fused agg) ====
        def edge_tile(tab, piece, sd_t, agg_out_t):
            (t, kk, io, mo) = piece
            G = gpool.tile([128, DMAX, REC], BF, tag="G")
            nc.gpsimd.dma_gather(
                out_ap=G[:, 0:kk, :], in_ap=tab[:, :],
                idxs_ap=idx_all[:, io:io + 8 * kk],
                num_idxs=128 * kk, num_idxs_reg=128 * kk, elem_size=REC,
                single_packet=False)
            # attention weights [128, kk]
            w = spool.tile([128, DMAX], FP, tag="w")
            nc.vector.tensor_tensor(w[:, 0:kk], G[:, 0:kk, HID],
                                    sd_t[:, t:t + 1].broadcast_to([128, kk]),
                                    ALU.add)
            nc.vector.scalar_tensor_tensor(
                w[:, 0:kk], w[:, 0:kk], 0.2, w[:, 0:kk], ALU.mult, ALU.max)
            nc.scalar.activation(w[:, 0:kk], w[:, 0:kk], AF.Exp)
            wm = spool.tile([128, DMAX, 1], BF, tag="wm")
            nc.vector.tensor_tensor(wm[:, 0:kk, 0], w[:, 0:kk],
                                    mask_all[:, mo:mo + kk], ALU.mult)
            dn = spool.tile([128, 1], FP, tag="dn")
            nc.vector.tensor_reduce(dn[:], wm[:, 0:kk, 0],
                                    axis=mybir.AxisListType.X, op=ALU.add)
            # weighted records + strided-axis reduction over slots
            WG = wgpool.tile([128, DMAX, HID], BF, tag="WG")
            nc.vector.tensor_tensor(
                WG[:, 0:kk, :], G[:, 0:kk, 0:HID],
                wm[:, 0:kk, :].broadcast_to([128, kk, HID]), ALU.mult)
            agg = vpool.tile([128, HID], FP, tag="agg")
            nc.vector.tensor_reduce(
                agg[:], WG[:, 0:kk, :].rearrange("p k f -> p f k"),
                axis=mybir.AxisListType.X, op=ALU.add)
            dn2 = spool.tile([128, 1], FP, tag="dn2")
            nc.vector.tensor_scalar_max(dn2[:], dn[:], EPS)
            rcp = spool.tile([128, 1], FP, tag="rcp")
            nc.vector.reciprocal(rcp[:], dn2[:])
            nc.vector.tensor_scalar_mul(agg_out_t, agg[:], rcp[:, 0:1])

        # ---- layer 1 edge + layer 2 node, interleaved per chunk
        for j in range(N_CHUNK):
            for t in range(CTILES * j, CTILES * (j + 1)):
                edge_tile(tab1, pieces[t], sd1, agg_all[:, t, :])
            # batched gelu for this chunk (single table load per chunk)
            nc.scalar.activation(
                t2_all[:, CTILES * j:CTILES * (j + 1), :].rearrange(
                    "p t f -> p (t f)"),
                agg_all[:, CTILES * j:CTILES * (j + 1), :].rearrange(
                    "p t f -> p (t f)"),
                AF.Gelu_apprx_tanh)
            for t in range(CTILES * j, CTILES * (j + 1)):
                tps = ppool.tile([128, 128], BF, tag="tr")
                nc.tensor.transpose(tps[:], t2_all[:, t, :], ident[:])
                tsb = iopool.tile([128, 128], BF, tag="t2T")
                nc.vector.tensor_copy(tsb[:], tps[:])
                node_phase(t, [(tsb, W2sb[:])], b2r, amats["a2s"],
                           amats["a2d"], sd2, tab2_sh)
            ag_chunk(tab2_sh, tab2, j)

        if phases == "l1":
            z0 = cpool.tile([8, HID + 1], FP, tag="zero")
            nc.vector.memset(z0[:], 0.0)
            nc.sync.dma_start(out_sh.ap(), z0[:])
            nc.sync.dma_start(gm_sh.ap(), z0[:])
            return

        # ---- layer 2 edge
        for t in range(TILES):
            edge_tile(tab2, pieces[t], sd2, agg2_all[:, t, :])
        for t in range(TILES):
            scr = vpool.tile([128, HID], FP, tag="e_scr")
            nc.vector.tensor_tensor(scr[:], agg2_all[:, t, :],
                                    agg2_all[:, t, :], ALU.mult)
            nc.vector.tensor_reduce(n2_all[:, t:t + 1], scr[:],
                                    axis=mybir.AxisListType.X, op=ALU.add)
        # batched expmap scalars: nn, sinh(n)/n
        nn_e = sc32a
        nc.scalar.sqrt(nn_e[:], n2_all[:])                      # table load
        ep = sc32b
        nc.scalar.activation(ep[:], nn_e[:], AF.Exp)            # table load
        em = sc32c
        nc.scalar.activation(em[:], nn_e[:], AF.Exp, scale=-1.0)
        sh = sc32b                                              # reuse ep slot
        nc.vector.tensor_tensor(sh[:], ep[:], em[:], ALU.subtract)
        nm = sc32c
        nc.vector.tensor_scalar_max(nm[:], nn_e[:], EPS)
        rn_e = sc32d
        nc.vector.reciprocal(rn_e[:], nm[:])
        cf_e = sc32b
        nc.vector.tensor_tensor(cf_e[:], sh[:], rn_e[:], ALU.mult)
        nc.vector.tensor_scalar_mul(cf_e[:], cf_e[:], 0.5)
        hn2_all = sc32c
        for t in range(TILES):
            nc.scalar.mul(h2_all[:, t, 1:HID + 1], agg2_all[:, t, :],
                          cf_e[:, t:t + 1])
            scr = vpool.tile([128, HID], FP, tag="e_scr2")
            nc.vector.tensor_tensor(scr[:], h2_all[:, t, 1:HID + 1],
                                    h2_all[:, t, 1:HID + 1], ALU.mult)
            nc.vector.tensor_reduce(hn2_all[:, t:t + 1], scr[:],
                                    axis=mybir.AxisListType.X, op=ALU.add)
        # h0 = sqrt(1 + |hs|^2), strided write into h2_all[:, :, 0]
        nc.scalar.activation(h2_all[:, :, 0], hn2_all[:], AF.Sqrt, bias=1.0)

        gm_ps = gmpool.tile([8, HID + 1], FP, tag="gmA")
        g_ps = gmpool.tile([8, HID + 1], FP, tag="gmB")
        for t in range(TILES):
            nc.tensor.matmul(gm_ps[:], ind_all[:, 16 * t:16 * t + 8],
                             h2_all[:, t, :], start=(t == 0),
                             stop=(t == TILES - 1))
            nc.tensor.matmul(g_ps[:], ind_all[:, 16 * t + 8:16 * (t + 1)],
                             h2_all[:, t, :], start=(t == 0),
                             stop=(t == TILES - 1))

        # ================= readout =================
        g = cpool.tile([8, HID + 1], FP, tag="f_g")
        nc.vector.tensor_copy(g[:], g_ps[:])
        ave = cpool.tile([8, HID + 1], FP)
        nc.scalar.mul(ave[:], gm_ps[:], 1.0 / 512.0)
        q = cpool.tile([8, 1], FP, tag="f_q")
        scr = vpool.tile([8, HID], FP, tag="f_scr")
        nc.vector.tensor_tensor(scr[:], ave[:, 1:HID + 1],
                                ave[:, 1:HID + 1], ALU.mult)
        nc.vector.tensor_reduce(q[:], scr[:],
                                axis=mybir.AxisListType.X, op=ALU.add)
        t0s = cpool.tile([8, 1], FP, tag="f_t0s")
        nc.vector.tensor_tensor(t0s[:], ave[:, 0:1], ave[:, 0:1], ALU.mult)
        dif = cpool.tile([8, 1], FP, tag="f_dif")
        nc.vector.tensor_tensor(dif[:], t0s[:], q[:], ALU.subtract)
        nc.vector.tensor_scalar_max(dif[:], dif[:], 1e-8)
        dsq = cpool.tile([8, 1], FP, tag="f_dsq")
        nc.scalar.sqrt(dsq[:], dif[:])
        rr = cpool.tile([8, 1], FP, tag="f_rr")
        nc.vector.reciprocal(rr[:], dsq[:])
        gm = cpool.tile([8, HID + 1], FP, tag="f_gm")
        nc.scalar.mul(gm[:], ave[:], rr[:, 0:1])
        nc.sync.dma_start(gm_sh.ap(), gm[:])

        # y = g @ W_lin
        gT_ps = ppool.tile([128, 8], FP, tag="tr")
        nc.tensor.transpose(gT_ps[:], g[:, 0:128], ident8[:])
        gT = cpool.tile([128, 8], FP, tag="f_gT")
        nc.vector.tensor_copy(gT[:], gT_ps[:])
        gl_ps = ppool1.tile([1, 8], FP, tag="tr2")
        nc.tensor.transpose(gl_ps[:], g[:, 128:129], ident8[:])
        gl = cpool.tile([1, 8], FP, tag="f_gl")
        nc.vector.tensor_copy(gl[:], gl_ps[:])
        y_ps = ppool1.tile([8, HID + 1], FP, tag="y")
        nc.tensor.matmul(y_ps[:], gT[:], Wlin[:], start=True, stop=False)
        nc.tensor.matmul(y_ps[:], gl[:], Wlin_l[:], start=False, stop=True)
        y = cpool.tile([8, HID + 1], FP, tag="f_y")
        nc.vector.tensor_copy(y[:], y_ps[:])

        ls_ps = ppool1.tile([8, 1], FP, tag="tr2")
        ones8 = cpool.tile([1, 8], FP, tag="f_ones8")
        nc.vector.memset(ones8[:], 1.0)
        nc.tensor.matmul(ls_ps[:], ones8[:], lsc[:], start=True, stop=True)
        lsb = cpool.tile([8, 1], FP, tag="f_lsb")
        nc.vector.tensor_copy(lsb[:], ls_ps[:])

        sig = cpool.tile([8, 1], FP, tag="f_sig")
        nc.scalar.activation(sig[:], y[:, 0:1], AF.Sigmoid)
        tme = cpool.tile([8, 1], FP, tag="f_tme")
        nc.vector.tensor_tensor(tme[:], sig[:], lsb[:], ALU.mult)
        nc.vector.tensor_scalar_add(tme[:], tme[:], 1.1)
        s2 = cpool.tile([8, 1], FP, tag="f_s2")
        scr2 = vpool.tile([8, HID], FP, tag="f_scr2")
        nc.vector.tensor_tensor(scr2[:], y[:, 1:HID + 1],
                                y[:, 1:HID + 1], ALU.mult)
        nc.vector.tensor_reduce(s2[:], scr2[:],
                                axis=mybir.AxisListType.X, op=ALU.add)
        nc.vector.tensor_scalar_max(s2[:], s2[:], 1e-8)
        rs2 = cpool.tile([8, 1], FP, tag="f_rs2")
        nc.vector.reciprocal(rs2[:], s2[:])
        tm1 = cpool.tile([8, 1], FP, tag="f_tm1")
        nc.vector.scalar_tensor_tensor(tm1[:], tme[:], 1.0, tme[:],
                                       ALU.mult, ALU.mult)
        nc.vector.tensor_scalar_add(tm1[:], tm1[:], -1.0)
        fac2 = cpool.tile([8, 1], FP, tag="f_fac2")
        nc.vector.tensor_tensor(fac2[:], tm1[:], rs2[:], ALU.mult)
        fac = cpool.tile([8, 1], FP, tag="f_fac")
        nc.scalar.sqrt(fac[:], fac2[:])
        outt = cpool.tile([8, HID + 1], FP, tag="f_out")
        nc.vector.tensor_copy(outt[:, 0:1], tme[:])
        nc.scalar.mul(outt[:, 1:HID + 1], y[:, 1:HID + 1], fac[:, 0:1])
        nc.sync.dma_start(out_sh.ap(), outt[:])


_CACHE = {}


def _get_compiled(edge_index):
    key = hash(np.asarray(edge_index).tobytes())
    if key not in _CACHE:
        pieces, CI, CM, per_core = _preprocess(edge_index)
        nc = _build(pieces, CI, CM)
        _CACHE[key] = (nc, per_core)
    return _CACHE[key]


def _make_in_maps(x, per_core, W1, b1, a1_src, a1_dst, W2, b2, a2_src,
                  a2_dst, W_lin, lin_scale):
    in_maps = []
    for c in range(N_CORES):
        pc = per_core[c]
        xp = x[SHARD * c + pc["perm"], :]
        in_maps.append(dict(
            x_perm=np.ascontiguousarray(xp),
            W1=np.asarray(W1, np.float32),
            W2=np.asarray(W2, np.float32),
            b1=np.asarray(b1, np.float32).reshape(1, HID),
            b2=np.asarray(b2, np.float32).reshape(1, HID),
            a1s=np.asarray(a1_src, np.float32).reshape(1, HID),
            a1d=np.asarray(a1_dst, np.float32).reshape(1, HID),
            a2s=np.asarray(a2_src, np.float32).reshape(1, HID),
            a2d=np.asarray(a2_dst, np.float32).reshape(1, HID),
            W_lin=np.asarray(W_lin, np.float32),
            lin_scale=np.asarray(lin_scale, np.float32).reshape(1, 1),
            idx=pc["idx"], mask=pc["mask"], ind=pc["ind"],
        ))
    return in_maps


def kernel(x, edge_index, batch_size, W1, b1, a1_src, a1_dst,
           W2, b2, a2_src, a2_dst, W_lin, lin_scale, _trace=False):
    x = np.asarray(x, np.float32)
    assert int(batch_size) == BATCH
    nc, per_core = _get_compiled(edge_index)
    in_maps = _make_in_maps(x, per_core, W1, b1, a1_src, a1_dst, W2, b2,
                            a2_src, a2_dst, W_lin, lin_scale)
    res = run_bass_kernel_spmd(nc, in_maps, core_ids=list(range(N_CORES)),
                               trace=_trace)
    out = np.concatenate([res.results[c]["out_shard"]
                          for c in range(N_CORES)], 0)
    gm = np.concatenate([res.results[c]["gm_shard"]
                         for c in range(N_CORES)], 0)
    if _trace:
        kernel.last_exec_time_ns = res.exec_time_ns
        kernel.last_results = res
    return (out, gm)


kernel.last_exec_time_ns = None


# revision 6
# speedup vs baseline: 1.5396x; 1.5306x over previous
"""LorentzGNN (2x Lorentz-GAT + readout) Trainium2 kernel, 8 NeuronCores.

Strategy (graph/data parallel, hardcoded from the sharding hint):
  - Core c owns dst nodes [4096c, 4096(c+1)) = 8 whole graphs of 512 nodes.
  - Within a shard, nodes are renumbered by degree (descending) so each
    128-node tile has a uniform padded-CSR depth D_t (max degree in tile).
  - Per layer: sharded node phase computes a 256-el bf16 record per node
    [z(0:128) | s_src(128) | pad], written to a DRAM table shard;
    AllGather (2 chunks, overlapped with compute) makes the full table
    visible to every core.
  - Edge phase: ONE dma_gather per dst-tile pulls the src-records of all
    incident edges into [128 dst-partitions, D_t slots, 256]; attention
    weights are computed as [128, D_t] ops, applied with a single big
    elementwise multiply, and reduced over slots with one strided-axis
    tensor_reduce (no per-slot MAC chain).
  - expmap0/projx/logmap0 between layers cancels analytically, so layer-2
    tangent input is just gelu(agg1).
  - Readout (centroid + g-rows + LorentzLinear) is computed on-device per
    core for its 8 graphs; host concatenates the [8,129] shards.
"""
import os
import sys
import copy
import time

sys.path.insert(0, "/opt/trn_rl_repo")

import numpy as np

import concourse.bacc as bacc
import concourse.tile as tile
import concourse.bass as bass
from concourse import mybir, masks
from concourse.bass_utils import run_bass_kernel_spmd

FP = mybir.dt.float32
BF = mybir.dt.bfloat16
AF = mybir.ActivationFunctionType
ALU = mybir.AluOpType

N_NODES = 32768
N_EDGES = 524288
FT_IN = 256
HID = 128
BATCH = 64
N_CORES = 8
SHARD = N_NODES // N_CORES      # 4096
TILES = SHARD // 128            # 32
N_CHUNK = 2                     # AllGather chunks per layer
CTILES = TILES // N_CHUNK       # tiles per chunk
REC = 256                       # record: [z(0:128) | s_src(128) | pad], bf16
KCH = 17                        # max slots per gather piece
EPS = 1e-7


# ---------------------------------------------------------------------------
# walrus in this container supports only ONE sync-wait per instruction;
# split extras onto standalone EventSemaphore instructions (same engine,
# immediately before -> program order preserves semantics).
def _split_waits(nc, max_waits=1):
    f = nc.m.functions[0]
    template = None
    for blk in f.blocks:
        for ins in blk.instructions:
            if type(ins).__name__ == "InstEventSemaphore":
                template = ins
                break
        if template is not None:
            break
    assert template is not None
    uid = 0
    for blk in f.blocks:
        new_list = []
        changed = False
        for ins in blk.instructions:
            si = ins.sync_info
            waits = list(si.on_wait) if si is not None else []
            if len(waits) > max_waits:
                keep = waits[-max_waits:]
                for w in waits[: len(waits) - max_waits]:
                    ev = copy.deepcopy(template)
                    ev.name = f"bass_split_wait_{uid}"
                    uid += 1
                    ev.engine = ins.engine
                    nsi = copy.deepcopy(si)
                    nsi.on_wait = [w]
                    nsi.on_update = []
                    ev.sync_info = nsi
                    new_list.append(ev)
                nsi2 = copy.deepcopy(si)
                nsi2.on_wait = keep
                ins.sync_info = nsi2
                changed = True
            new_list.append(ins)
        if changed:
            blk.instructions = new_list


# ---------------------------------------------------------------------------
# Host-side graph preprocessing: sharding, degree-sort renumbering,
# whole-tile padded-CSR gather indices, masks, per-tile readout indicators.
#
# Global table row for (core c, local degree-sorted row l):
#   chunk = l // (SHARD//N_CHUNK); row = chunk*(N_NODES//N_CHUNK)
#           + (SHARD//N_CHUNK)*c + (l % (SHARD//N_CHUNK))
# so an AllGather of chunk j (concat of all cores' chunk-j shard slices)
# lands records exactly at their global rows.
def _preprocess(edge_index):
    dst = np.asarray(edge_index[0], np.int64)
    src = np.asarray(edge_index[1], np.int64)
    CH_SH = SHARD // N_CHUNK          # local rows per chunk
    CH_GL = N_NODES // N_CHUNK        # global rows per chunk

    perms = []       # per core: local row j -> original local node
    invperms = []    # per core: original local node -> local row
    degs = []
    for c in range(N_CORES):
        sel = (dst >= SHARD * c) & (dst < SHARD * (c + 1))
        dloc = dst[sel] - SHARD * c
        deg = np.bincount(dloc, minlength=SHARD)
        order = np.argsort(-deg, kind="stable")
        inv = np.empty(SHARD, np.int64)
        inv[order] = np.arange(SHARD)
        perms.append(order)
        invperms.append(inv)
        degs.append(deg)

    # renumbered global table row of original node s (chunk-major layout)
    renum = np.empty(N_NODES, np.int64)
    for c in range(N_CORES):
        ell = invperms[c]
        renum[SHARD * c: SHARD * (c + 1)] = (
            (ell // CH_SH) * CH_GL + CH_SH * c + (ell % CH_SH))

    # uniform tile depths across cores
    Dt = np.zeros(TILES, np.int64)
    for c in range(N_CORES):
        sd = degs[c][perms[c]]                      # sorted degrees
        for t in range(TILES):
            Dt[t] = max(Dt[t], sd[128 * t: 128 * (t + 1)].max())
    Dt = np.maximum(Dt, 1)

    # pieces: (tile, k0, kk, first, last, idx_off, mask_off), kk <= KCH
    pieces = []
    ioff = moff = 0
    for t in range(TILES):
        k0 = 0
        while k0 < Dt[t]:
            kk = int(min(KCH, Dt[t] - k0))
            pieces.append((t, k0, kk, k0 == 0, k0 + kk == int(Dt[t]),
                           ioff, moff))
            ioff += 8 * kk
            moff += kk
            k0 += kk
    CI, CM = ioff, moff

    # per-core CSR in renumbered order + idx/mask/indicator buffers
    per_core = []
    for c in range(N_CORES):
        sel = (dst >= SHARD * c) & (dst < SHARD * (c + 1))
        dloc = dst[sel] - SHARD * c
        sglob = src[sel]
        eorder = np.argsort(invperms[c][dloc], kind="stable")
        s_sorted = renum[sglob[eorder]]             # src table rows
        deg_r = degs[c][perms[c]]                   # degree per local row
        starts = np.zeros(SHARD + 1, np.int64)
        starts[1:] = np.cumsum(deg_r)

        idx_buf = np.zeros((128, CI), np.int16)
        mask_buf = np.zeros((128, CM), np.float32)
        for (t, k0, kk, _f, _l, io, mo) in pieces:
            lin = np.zeros(128 * kk, np.int64)
            msk = np.zeros((128, kk), np.float32)
            rows = 128 * t + np.arange(128)
            for j in range(128):
                r = rows[j]
                d = deg_r[r]
                lo, hi = k0, min(d, k0 + kk)
                if hi > lo:
                    e0 = starts[r] + lo
                    kks = np.arange(lo, hi) - k0
                    lin[kks * 128 + j] = s_sorted[e0: e0 + (hi - lo)]
                    msk[j, : hi - lo] = 1.0
            wrapped = lin.astype(np.int16).reshape(-1, 16).T   # [16, 8*kk]
            for g in range(8):
                idx_buf[16 * g: 16 * (g + 1), io: io + 8 * kk] = wrapped
            mask_buf[:, mo: mo + kk] = msk

        ind_buf = np.zeros((128, 16 * TILES), np.float32)
        for t in range(TILES):
            for j in range(128):
                orig = SHARD * c + perms[c][128 * t + j]
                gcol = orig // 512 - 8 * c
                ind_buf[j, 16 * t + gcol] = 1.0
                if orig % 512 == 0:
                    ind_buf[j, 16 * t + 8 + gcol] = 1.0

        per_core.append(dict(idx=idx_buf, mask=mask_buf, ind=ind_buf,
                             perm=perms[c]))

    return pieces, CI, CM, per_core


# ---------------------------------------------------------------------------
def _build(pieces, CI, CM):
    n_dev = 1 if os.environ.get("K_SINGLE") else N_CORES
    nc = bacc.Bacc("TRN2", target_bir_lowering=False, debug=False,
                   num_devices=n_dev, num_swdge_queues=4)
    I = {}
    I["x_perm"] = nc.dram_tensor("x_perm", [SHARD, FT_IN + 1], FP,
                                 kind="ExternalInput")
    I["W1"] = nc.dram_tensor("W1", [FT_IN, HID], FP, kind="ExternalInput")
    I["W2"] = nc.dram_tensor("W2", [HID, HID], FP, kind="ExternalInput")
    I["b1"] = nc.dram_tensor("b1", [1, HID], FP, kind="ExternalInput")
    I["b2"] = nc.dram_tensor("b2", [1, HID], FP, kind="ExternalInput")
    for nm in ("a1s", "a1d", "a2s", "a2d"):
        I[nm] = nc.dram_tensor(nm, [1, HID], FP, kind="ExternalInput")
    I["W_lin"] = nc.dram_tensor("W_lin", [HID + 1, HID + 1], FP,
                                kind="ExternalInput")
    I["lin_scale"] = nc.dram_tensor("lin_scale", [1, 1], FP,
                                    kind="ExternalInput")
    I["idx"] = nc.dram_tensor("idx", [128, CI], mybir.dt.int16,
                              kind="ExternalInput")
    I["mask"] = nc.dram_tensor("mask", [128, CM], FP, kind="ExternalInput")
    I["ind"] = nc.dram_tensor("ind", [128, 16 * TILES], FP,
                              kind="ExternalInput")
    out_sh = nc.dram_tensor("out_shard", [8, HID + 1], FP,
                            kind="ExternalOutput")
    gm_sh = nc.dram_tensor("gm_shard", [8, HID + 1], FP,
                           kind="ExternalOutput")

    REP = int(os.environ.get("K_REPEAT", "1"))
    with tile.TileContext(nc) as tc:
        for _ in range(REP):
            _trace(nc, tc, I, out_sh, gm_sh, pieces)
    nc.compile()
    _split_waits(nc)
    return nc


def _trace(nc, tc, I, out_sh, gm_sh, pieces):
    DMAX = max(kk for (_t, _k0, kk, _f, _l, _io, _mo) in pieces)
    with (
        tc.tile_pool(name="const", bufs=1) as cpool,
        tc.tile_pool(name="io", bufs=2) as iopool,
        tc.tile_pool(name="gat", bufs=6) as gpool,
        tc.tile_pool(name="wg", bufs=2) as wgpool,
        tc.tile_pool(name="vv", bufs=2) as vpool,
        tc.tile_pool(name="sm", bufs=4) as spool,
        tc.tile_pool(name="ps", bufs=2, space="PSUM") as ppool,
        tc.tile_pool(name="ps1", bufs=1, space="PSUM") as ppool1,
        tc.tile_pool(name="psg", bufs=1, space="PSUM") as gmpool,
        tc.tile_pool(name="dram", bufs=1, space="DRAM") as dpool,
    ):
        # ---- constants
        ident = cpool.tile([128, 128], BF)
        masks.make_identity(nc, ident[:])
        ident8 = cpool.tile([8, 8], FP)
        masks.make_identity(nc, ident8[:])
        ones_row = cpool.tile([1, 128], FP)
        nc.vector.memset(ones_row[:], 1.0)

        idx_all = cpool.tile([128, max(I["idx"].shape[1], 16)],
                             mybir.dt.int16)
        nc.sync.dma_start(idx_all[:, 0:I["idx"].shape[1]], I["idx"].ap())
        mask_all = cpool.tile([128, max(I["mask"].shape[1], 4)], BF)
        nc.gpsimd.dma_start(mask_all[:, 0:I["mask"].shape[1]],
                            I["mask"].ap())
        ind_all = cpool.tile([128, 16 * TILES], FP)
        nc.sync.dma_start(ind_all[:], I["ind"].ap())

        # weights as bf16 lhsT tiles (cast on ACT after f32 load)
        W1f = iopool.tile([128, 2, HID], FP, tag="w1f")
        nc.sync.dma_start(W1f[:, 0, :], I["W1"].ap()[0:128, :])
        nc.sync.dma_start(W1f[:, 1, :], I["W1"].ap()[128:256, :])
        W1sb = cpool.tile([128, 2, HID], BF)
        nc.vector.tensor_copy(W1sb[:].rearrange("p a h -> p (a h)"),
                              W1f[:].rearrange("p a h -> p (a h)"))
        W2f = iopool.tile([128, HID], FP, tag="w2f")
        nc.sync.dma_start(W2f[:], I["W2"].ap())
        W2sb = cpool.tile([128, HID], BF)
        nc.vector.tensor_copy(W2sb[:], W2f[:])
        b1r = cpool.tile([1, HID], FP)
        nc.sync.dma_start(b1r[:], I["b1"].ap())
        b2r = cpool.tile([1, HID], FP)
        nc.sync.dma_start(b2r[:], I["b2"].ap())
        Wlin = cpool.tile([128, HID + 1], FP)
        nc.sync.dma_start(Wlin[:], I["W_lin"].ap()[0:128, :])
        Wlin_l = cpool.tile([1, HID + 1], FP)
        nc.sync.dma_start(Wlin_l[:], I["W_lin"].ap()[128:129, :])
        lsc = cpool.tile([1, 1], FP)
        nc.sync.dma_start(lsc[:], I["lin_scale"].ap())

        # replicated a-vectors via PE outer product with ones
        amats = {}
        for nm in ("a1s", "a1d", "a2s", "a2d"):
            row = cpool.tile([1, HID], FP, tag=f"row_{nm}")
            nc.sync.dma_start(row[:], I[nm].ap())
            ps = ppool.tile([128, HID], FP, tag="tr")
            nc.tensor.matmul(ps[:], ones_row[:], row[:], start=True, stop=True)
            m = cpool.tile([128, HID], FP, tag=f"amat_{nm}")
            nc.vector.tensor_copy(m[:], ps[:])
            amats[nm] = m

        # persistent per-layer state
        sd1 = cpool.tile([128, TILES], FP)
        sd2 = cpool.tile([128, TILES], FP)
        agg_all = cpool.tile([128, TILES, HID], FP, tag="agg_all")
        t2_all = cpool.tile([128, TILES, HID], BF, tag="t2_all")
        agg2_all = cpool.tile([128, TILES, HID], FP, tag="agg2_all")
        h2_all = cpool.tile([128, TILES, HID + 1], FP, tag="h2_all")
        n2_all = cpool.tile([128, TILES], FP, tag="n2_all")
        sc32a = cpool.tile([128, TILES], FP, tag="sc32a")
        sc32b = cpool.tile([128, TILES], FP, tag="sc32b")
        sc32c = cpool.tile([128, TILES], FP, tag="sc32c")
        sc32d = cpool.tile([128, TILES], FP, tag="sc32d")
        xall = cpool.tile([128, TILES, FT_IN + 1], FP, tag="xall")

        tab1_sh = dpool.tile([SHARD, REC], BF)
        tab1 = dpool.tile([N_NODES, REC], BF)
        tab2_sh = dpool.tile([SHARD, REC], BF)
        tab2 = dpool.tile([N_NODES, REC], BF)

        CH_SH = SHARD // N_CHUNK
        CH_GL = N_NODES // N_CHUNK

        def ag_chunk(tab_sh, tab, j):
            if os.environ.get("K_SINGLE"):
                nc.sync.dma_start(
                    tab[CH_GL * j: CH_GL * j + CH_SH, :],
                    tab_sh[CH_SH * j: CH_SH * (j + 1), :])
            else:
                nc.gpsimd.collective_compute(
                    "AllGather", ALU.bypass,
                    replica_groups=[list(range(N_CORES))],
                    ins=[tab_sh[CH_SH * j: CH_SH * (j + 1), :].opt()],
                    outs=[tab[CH_GL * j: CH_GL * (j + 1), :].opt()])

        def node_phase(t, tanT_parts, brow, ams, amd, sd_t, tab_shard):
            """z = tan @ W + b into PSUM; record row + s_src/s_dst."""
            z_ps = ppool.tile([128, HID], FP, tag="z")
            for i, (tT, Wp) in enumerate(tanT_parts):
                nc.tensor.matmul(z_ps[:], tT[:], Wp, start=(i == 0),
                                 stop=False)
            nc.tensor.matmul(z_ps[:], ones_row[:], brow[:], start=False,
                             stop=True)
            stg = iopool.tile([128, REC], BF, tag="stg")
            nc.scalar.copy(stg[:, 0:HID], z_ps[:])
            scr = vpool.tile([128, HID], FP, tag="scr")
            nc.vector.tensor_tensor(scr[:], z_ps[:], ams[:], ALU.mult)
            ssf = spool.tile([128, 1], FP, tag="ssf")
            nc.vector.tensor_reduce(ssf[:], scr[:],
                                    axis=mybir.AxisListType.X, op=ALU.add)
            nc.vector.tensor_copy(stg[:, HID:HID + 1], ssf[:])
            scr2 = vpool.tile([128, HID], FP, tag="scr2")
            nc.vector.tensor_tensor(scr2[:], z_ps[:], amd[:], ALU.mult)
            nc.vector.tensor_reduce(sd_t[:, t:t + 1], scr2[:],
                                    axis=mybir.AxisListType.X, op=ALU.add)
            nc.sync.dma_start(tab_shard[128 * t:128 * (t + 1), :], stg[:])

        # ============ phase A1: logmap (batched ACT) + conv1 node part ====
        nc.sync.dma_start(
            xall[:], I["x_perm"].ap().rearrange("(t p) f -> p t f", p=128))
        for t in range(TILES):
            scr = vpool.tile([128, FT_IN], FP, tag="scrA")
            nc.vector.tensor_tensor(scr[:], xall[:, t, 1:FT_IN + 1],
                                    xall[:, t, 1:FT_IN + 1], ALU.mult)
            nc.vector.tensor_reduce(n2_all[:, t:t + 1], scr[:],
                                    axis=mybir.AxisListType.X, op=ALU.add)
        nn_a = sc32a
        nc.scalar.sqrt(nn_a[:], n2_all[:])                      # 1 table load
        npx = sc32b
        nc.vector.tensor_tensor(npx[:], nn_a[:], xall[:, :, 0], ALU.add)
        lt = sc32c
        nc.scalar.activation(lt[:], npx[:], AF.Ln)              # 1 table load
        rn = sc32d
        nc.vector.reciprocal(rn[:], nn_a[:])
        cf_a = sc32b                                            # reuse
        nc.vector.tensor_tensor(cf_a[:], lt[:], rn[:], ALU.mult)

        for j in range(N_CHUNK):
            for t in range(CTILES * j, CTILES * (j + 1)):
                tan = iopool.tile([128, FT_IN], BF, tag="tan")
                nc.scalar.mul(tan[:], xall[:, t, 1:FT_IN + 1],
                              cf_a[:, t:t + 1])
                parts = []
                for h in range(2):
                    tps = ppool.tile([128, 128], BF, tag="tr")
                    nc.tensor.transpose(tps[:],
                                        tan[:, 128 * h:128 * (h + 1)],
                                        ident[:])
                    tsb = iopool.tile([128, 128], BF, tag=f"tT{h}")
                    nc.vector.tensor_copy(tsb[:], tps[:])
                    parts.append((tsb, W1sb[:, h, :]))
                node_phase(t, parts, b1r, amats["a1s"], amats["a1d"],
                           sd1, tab1_sh)
            ag_chunk(tab1_sh, tab1, j)

        phases = os.environ.get("K_PHASES", "full")
        if phases == "a1":
            z0 = cpool.tile([8, HID + 1], FP, tag="zero")
            nc.vector.memset(z0[:], 0.0)
            nc.sync.dma_start(out_sh.ap(), z0[:])
            nc.sync.dma_start(gm_sh.ap(), z0[:])
            return

        # ======= edge phase (pieces of <=KCH slots; 4 SWDGE queues) ====
        qctr = [0]

        def edge_piece(tab, piece, sd_t, agg_out_t, state):
            (t, k0, kk, first, last, io, mo) = piece
            G = gpool.tile([128, DMAX, REC], BF, tag="G")
            nc.gpsimd.dma_gather(
                out_ap=G[:, 0:kk, :], in_ap=tab[:, :],
                idxs_ap=idx_all[:, io:io + 8 * kk],
                num_idxs=128 * kk, num_idxs_reg=128 * kk, elem_size=REC,
                single_packet=False, queue_num=qctr[0] % 4)
            qctr[0] += 1
            # attention weights [128, kk]
            w = spool.tile([128, DMAX], FP, tag="w")
            nc.vector.tensor_tensor(w[:, 0:kk], G[:, 0:kk, HID],
                                    sd_t[:, t:t + 1].broadcast_to([128, kk]),
                                    ALU.add)
            nc.vector.scalar_tensor_tensor(
                w[:, 0:kk], w[:, 0:kk], 0.2, w[:, 0:kk], ALU.mult, ALU.max)
            nc.scalar.activation(w[:, 0:kk], w[:, 0:kk], AF.Exp)
            wm = spool.tile([128, DMAX, 1], BF, tag="wm")
            nc.vector.tensor_tensor(wm[:, 0:kk, 0], w[:, 0:kk],
                                    mask_all[:, mo:mo + kk], ALU.mult)
            if first:
                dn = spool.tile([128, 1], FP, tag="dn")
                state["dn"] = dn
                nc.vector.tensor_reduce(dn[:], wm[:, 0:kk, 0],
                                        axis=mybir.AxisListType.X, op=ALU.add)
            else:
                dnp = spool.tile([128, 1], FP, tag="dnp")
                nc.vector.tensor_reduce(dnp[:], wm[:, 0:kk, 0],
                                        axis=mybir.AxisListType.X, op=ALU.add)
                nc.vector.tensor_tensor(state["dn"][:], state["dn"][:],
                                        dnp[:], ALU.add)
            # weighted records + strided-axis reduction over slots
            WG = wgpool.tile([128, DMAX, HID], BF, tag="WG")
            nc.vector.tensor_tensor(
                WG[:, 0:kk, :], G[:, 0:kk, 0:HID],
                wm[:, 0:kk, :].broadcast_to([128, kk, HID]), ALU.mult)
            if first:
                agg = vpool.tile([128, HID], FP, tag="agg")
                state["agg"] = agg
                nc.vector.tensor_reduce(
                    agg[:], WG[:, 0:kk, :].rearrange("p k f -> p f k"),
                    axis=mybir.AxisListType.X, op=ALU.add)
            else:
                aggp = vpool.tile([128, HID], FP, tag="aggp")
                nc.vector.tensor_reduce(
                    aggp[:], WG[:, 0:kk, :].rearrange("p k f -> p f k"),
                    axis=mybir.AxisListType.X, op=ALU.add)
                nc.vector.tensor_tensor(state["agg"][:], state["agg"][:],
                                        aggp[:], ALU.add)
            if last:
                dn2 = spool.tile([128, 1], FP, tag="dn2")
                nc.vector.tensor_scalar_max(dn2[:], state["dn"][:], EPS)
                rcp = spool.tile([128, 1], FP, tag="rcp")
                nc.vector.reciprocal(rcp[:], dn2[:])
                nc.vector.tensor_tensor(
                    agg_out_t, state["agg"][:],
                    rcp[:, 0:1].broadcast_to([128, HID]), ALU.mult)

        def edge_tiles(tab, sd_t, agg_all_t, t_lo, t_hi):
            state = {}
            for piece in pieces:
                t = piece[0]
                if t_lo <= t < t_hi:
                    edge_piece(tab, piece, sd_t, agg_all_t[:, t, :], state)

        # ---- layer 1 edge + layer 2 node, interleaved per chunk
        for j in range(N_CHUNK):
            edge_tiles(tab1, sd1, agg_all, CTILES * j, CTILES * (j + 1))
            # batched gelu for this chunk (single table load per chunk)
            nc.scalar.activation(
                t2_all[:, CTILES * j:CTILES * (j + 1), :].rearrange(
                    "p t f -> p (t f)"),
                agg_all[:, CTILES * j:CTILES * (j + 1), :].rearrange(
                    "p t f -> p (t f)"),
                AF.Gelu_apprx_tanh)
            for t in range(CTILES * j, CTILES * (j + 1)):
                tps = ppool.tile([128, 128], BF, tag="tr")
                nc.tensor.transpose(tps[:], t2_all[:, t, :], ident[:])
                tsb = iopool.tile([128, 128], BF, tag="t2T")
                nc.vector.tensor_copy(tsb[:], tps[:])
                node_phase(t, [(tsb, W2sb[:])], b2r, amats["a2s"],
                           amats["a2d"], sd2, tab2_sh)
            ag_chunk(tab2_sh, tab2, j)

        if phases == "l1":
            z0 = cpool.tile([8, HID + 1], FP, tag="zero")
            nc.vector.memset(z0[:], 0.0)
            nc.sync.dma_start(out_sh.ap(), z0[:])
            nc.sync.dma_start(gm_sh.ap(), z0[:])
            return

        # ---- layer 2 edge
        edge_tiles(tab2, sd2, agg2_all, 0, TILES)
        for t in range(TILES):
            scr = vpool.tile([128, HID], FP, tag="e_scr")
            nc.vector.tensor_tensor(scr[:], agg2_all[:, t, :],
                                    agg2_all[:, t, :], ALU.mult)
            nc.vector.tensor_reduce(n2_all[:, t:t + 1], scr[:],
                                    axis=mybir.AxisListType.X, op=ALU.add)
        # batched expmap scalars: nn, sinh(n)/n
        nn_e = sc32a
        nc.scalar.sqrt(nn_e[:], n2_all[:])                      # table load
        ep = sc32b
        nc.scalar.activation(ep[:], nn_e[:], AF.Exp)            # table load
        em = sc32c
        nc.scalar.activation(em[:], nn_e[:], AF.Exp, scale=-1.0)
        sh = sc32b                                              # reuse ep slot
        nc.vector.tensor_tensor(sh[:], ep[:], em[:], ALU.subtract)
        nm = sc32c
        nc.vector.tensor_scalar_max(nm[:], nn_e[:], EPS)
        rn_e = sc32d
        nc.vector.reciprocal(rn_e[:], nm[:])
        cf_e = sc32b
        nc.vector.tensor_tensor(cf_e[:], sh[:], rn_e[:], ALU.mult)
        nc.vector.tensor_scalar_mul(cf_e[:], cf_e[:], 0.5)
        hn2_all = sc32c
        for t in range(TILES):
            nc.scalar.mul(h2_all[:, t, 1:HID + 1], agg2_all[:, t, :],
                          cf_e[:, t:t + 1])
            scr = vpool.tile([128, HID], FP, tag="e_scr2")
            nc.vector.tensor_tensor(scr[:], h2_all[:, t, 1:HID + 1],
                                    h2_all[:, t, 1:HID + 1], ALU.mult)
            nc.vector.tensor_reduce(hn2_all[:, t:t + 1], scr[:],
                                    axis=mybir.AxisListType.X, op=ALU.add)
        # h0 = sqrt(1 + |hs|^2), strided write into h2_all[:, :, 0]
        nc.scalar.activation(h2_all[:, :, 0], hn2_all[:], AF.Sqrt, bias=1.0)

        gm_ps = gmpool.tile([8, HID + 1], FP, tag="gmA")
        g_ps = gmpool.tile([8, HID + 1], FP, tag="gmB")
        for t in range(TILES):
            nc.tensor.matmul(gm_ps[:], ind_all[:, 16 * t:16 * t + 8],
                             h2_all[:, t, :], start=(t == 0),
                             stop=(t == TILES - 1))
            nc.tensor.matmul(g_ps[:], ind_all[:, 16 * t + 8:16 * (t + 1)],
                             h2_all[:, t, :], start=(t == 0),
                             stop=(t == TILES - 1))

        # ================= readout =================
        g = cpool.tile([8, HID + 1], FP, tag="f_g")
        nc.vector.tensor_copy(g[:], g_ps[:])
        ave = cpool.tile([8, HID + 1], FP)
        nc.scalar.mul(ave[:], gm_ps[:], 1.0 / 512.0)
        q = cpool.tile([8, 1], FP, tag="f_q")
        scr = vpool.tile([8, HID], FP, tag="f_scr")
        nc.vector.tensor_tensor(scr[:], ave[:, 1:HID + 1],
                                ave[:, 1:HID + 1], ALU.mult)
        nc.vector.tensor_reduce(q[:], scr[:],
                                axis=mybir.AxisListType.X, op=ALU.add)
        t0s = cpool.tile([8, 1], FP, tag="f_t0s")
        nc.vector.tensor_tensor(t0s[:], ave[:, 0:1], ave[:, 0:1], ALU.mult)
        dif = cpool.tile([8, 1], FP, tag="f_dif")
        nc.vector.tensor_tensor(dif[:], t0s[:], q[:], ALU.subtract)
        nc.vector.tensor_scalar_max(dif[:], dif[:], 1e-8)
        dsq = cpool.tile([8, 1], FP, tag="f_dsq")
        nc.scalar.sqrt(dsq[:], dif[:])
        rr = cpool.tile([8, 1], FP, tag="f_rr")
        nc.vector.reciprocal(rr[:], dsq[:])
        gm = cpool.tile([8, HID + 1], FP, tag="f_gm")
        nc.scalar.mul(gm[:], ave[:], rr[:, 0:1])
        nc.sync.dma_start(gm_sh.ap(), gm[:])

        # y = g @ W_lin
        gT_ps = ppool.tile([128, 8], FP, tag="tr")
        nc.tensor.transpose(gT_ps[:], g[:, 0:128], ident8[:])
        gT = cpool.tile([128, 8], FP, tag="f_gT")
        nc.vector.tensor_copy(gT[:], gT_ps[:])
        gl_ps = ppool1.tile([1, 8], FP, tag="tr2")
        nc.tensor.transpose(gl_ps[:], g[:, 128:129], ident8[:])
        gl = cpool.tile([1, 8], FP, tag="f_gl")
        nc.vector.tensor_copy(gl[:], gl_ps[:])
        y_ps = ppool1.tile([8, HID + 1], FP, tag="y")
        nc.tensor.matmul(y_ps[:], gT[:], Wlin[:], start=True, stop=False)
        nc.tensor.matmul(y_ps[:], gl[:], Wlin_l[:], start=False, stop=True)
        y = cpool.tile([8, HID + 1], FP, tag="f_y")
        nc.vector.tensor_copy(y[:], y_ps[:])

        ls_ps = ppool1.tile([8, 1], FP, tag="tr2")
        ones8 = cpool.tile([1, 8], FP, tag="f_ones8")
        nc.vector.memset(ones8[:], 1.0)
        nc.tensor.matmul(ls_ps[:], ones8[:], lsc[:], start=True, stop=True)
        lsb = cpool.tile([8, 1], FP, tag="f_lsb")
        nc.vector.tensor_copy(lsb[:], ls_ps[:])

        sig = cpool.tile([8, 1], FP, tag="f_sig")
        nc.scalar.activation(sig[:], y[:, 0:1], AF.Sigmoid)
        tme = cpool.tile([8, 1], FP, tag="f_tme")
        nc.vector.tensor_tensor(tme[:], sig[:], lsb[:], ALU.mult)
        nc.vector.tensor_scalar_add(tme[:], tme[:], 1.1)
        s2 = cpool.tile([8, 1], FP, tag="f_s2")
        scr2 = vpool.tile([8, HID], FP, tag="f_scr2")
        nc.vector.tensor_tensor(scr2[:], y[:, 1:HID + 1],
                                y[:, 1:HID + 1], ALU.mult)
        nc.vector.tensor_reduce(s2[:], scr2[:],
                                axis=mybir.AxisListType.X, op=ALU.add)
        nc.vector.tensor_scalar_max(s2[:], s2[:], 1e-8)
        rs2 = cpool.tile([8, 1], FP, tag="f_rs2")
        nc.vector.reciprocal(rs2[:], s2[:])
        tm1 = cpool.tile([8, 1], FP, tag="f_tm1")
        nc.vector.scalar_tensor_tensor(tm1[:], tme[:], 1.0, tme[:],
                                       ALU.mult, ALU.mult)
        nc.vector.tensor_scalar_add(tm1[:], tm1[:], -1.0)
        fac2 = cpool.tile([8, 1], FP, tag="f_fac2")
        nc.vector.tensor_tensor(fac2[:], tm1[:], rs2[:], ALU.mult)
        fac = cpool.tile([8, 1], FP, tag="f_fac")
        nc.scalar.sqrt(fac[:], fac2[:])
        outt = cpool.tile([8, HID + 1], FP, tag="f_out")
        nc.vector.tensor_copy(outt[:, 0:1], tme[:])
        nc.scalar.mul(outt[:, 1:HID + 1], y[:, 1:HID + 1], fac[:, 0:1])
        nc.sync.dma_start(out_sh.ap(), outt[:])


_CACHE = {}


def _get_compiled(edge_index):
    key = hash(np.asarray(edge_index).tobytes())
    if key not in _CACHE:
        pieces, CI, CM, per_core = _preprocess(edge_index)
        nc = _build(pieces, CI, CM)
        _CACHE[key] = (nc, per_core)
    return _CACHE[key]


def _make_in_maps(x, per_core, W1, b1, a1_src, a1_dst, W2, b2, a2_src,
                  a2_dst, W_lin, lin_scale):
    in_maps = []
    for c in range(N_CORES):
        pc = per_core[c]
        xp = x[SHARD * c + pc["perm"], :]
        in_maps.append(dict(
            x_perm=np.ascontiguousarray(xp),
            W1=np.asarray(W1, np.float32),
            W2=np.asarray(W2, np.float32),
            b1=np.asarray(b1, np.float32).reshape(1, HID),
            b2=np.asarray(b2, np.float32).reshape(1, HID),
            a1s=np.asarray(a1_src, np.float32).reshape(1, HID),
            a1d=np.asarray(a1_dst, np.float32).reshape(1, HID),
            a2s=np.asarray(a2_src, np.float32).reshape(1, HID),
            a2d=np.asarray(a2_dst, np.float32).reshape(1, HID),
            W_lin=np.asarray(W_lin, np.float32),
            lin_scale=np.asarray(lin_scale, np.float32).reshape(1, 1),
            idx=pc["idx"], mask=pc["mask"], ind=pc["ind"],
        ))
    return in_maps


def kernel(x, edge_index, batch_size, W1, b1, a1_src, a1_dst,
           W2, b2, a2_src, a2_dst, W_lin, lin_scale, _trace=False):
    x = np.asarray(x, np.float32)
    assert int(batch_size) == BATCH
    nc, per_core = _get_compiled(edge_index)
    in_maps = _make_in_maps(x, per_core, W1, b1, a1_src, a1_dst, W2, b2,
                            a2_src, a2_dst, W_lin, lin_scale)
    res = run_bass_kernel_spmd(nc, in_maps, core_ids=list(range(N_CORES)),
                               trace=_trace)
    out = np.concatenate([res.results[c]["out_shard"]
                          for c in range(N_CORES)], 0)
    gm = np.concatenate([res.results[c]["gm_shard"]
                         for c in range(N_CORES)], 0)
    if _trace:
        kernel.last_exec_time_ns = res.exec_time_ns
        kernel.last_results = res
    return (out, gm)


kernel.last_exec_time_ns = None


# revision 8
# speedup vs baseline: 1.7646x; 1.1461x over previous
"""LorentzGNN (2x Lorentz-GAT + readout) Trainium2 kernel, 8 NeuronCores.

Strategy (graph/data parallel, hardcoded from the sharding hint):
  - Core c owns dst nodes [4096c, 4096(c+1)) = 8 whole graphs of 512 nodes.
  - Within a shard, nodes are renumbered by degree (descending) so each
    128-node tile has a uniform padded-CSR depth D_t (max degree in tile).
  - Per layer: sharded node phase computes a 256-el bf16 record per node
    [z(0:128) | s_src(128) | pad], written to a DRAM table shard;
    AllGather (2 chunks, overlapped with compute) makes the full table
    visible to every core.
  - Edge phase: ONE dma_gather per dst-tile pulls the src-records of all
    incident edges into [128 dst-partitions, D_t slots, 256]; attention
    weights are computed as [128, D_t] ops, applied with a single big
    elementwise multiply, and reduced over slots with one strided-axis
    tensor_reduce (no per-slot MAC chain).
  - expmap0/projx/logmap0 between layers cancels analytically, so layer-2
    tangent input is just gelu(agg1).
  - Readout (centroid + g-rows + LorentzLinear) is computed on-device per
    core for its 8 graphs; host concatenates the [8,129] shards.
"""
import os
import sys
import copy
import time

sys.path.insert(0, "/opt/trn_rl_repo")

import numpy as np

import concourse.bacc as bacc
import concourse.tile as tile
import concourse.bass as bass
from concourse import mybir, masks
from concourse.bass_utils import run_bass_kernel_spmd

FP = mybir.dt.float32
BF = mybir.dt.bfloat16
AF = mybir.ActivationFunctionType
ALU = mybir.AluOpType

N_NODES = 32768
N_EDGES = 524288
FT_IN = 256
HID = 128
BATCH = 64
N_CORES = 8
SHARD = N_NODES // N_CORES      # 4096
TILES = SHARD // 128            # 32
N_CHUNK = 2                     # AllGather chunks per layer
CTILES = TILES // N_CHUNK       # tiles per chunk
REC = 256                       # record: [z(0:128) | s_src(128) | pad], bf16
KCH = 17                        # max slots per gather piece
EPS = 1e-7


# ---------------------------------------------------------------------------
# walrus in this container supports only ONE sync-wait per instruction;
# split extras onto standalone EventSemaphore instructions (same engine,
# immediately before -> program order preserves semantics).
def _split_waits(nc, max_waits=1):
    f = nc.m.functions[0]
    template = None
    for blk in f.blocks:
        for ins in blk.instructions:
            if type(ins).__name__ == "InstEventSemaphore":
                template = ins
                break
        if template is not None:
            break
    assert template is not None
    uid = 0
    for blk in f.blocks:
        new_list = []
        changed = False
        for ins in blk.instructions:
            si = ins.sync_info
            waits = list(si.on_wait) if si is not None else []
            if len(waits) > max_waits:
                keep = waits[-max_waits:]
                for w in waits[: len(waits) - max_waits]:
                    ev = copy.deepcopy(template)
                    ev.name = f"bass_split_wait_{uid}"
                    uid += 1
                    ev.engine = ins.engine
                    nsi = copy.deepcopy(si)
                    nsi.on_wait = [w]
                    nsi.on_update = []
                    ev.sync_info = nsi
                    new_list.append(ev)
                nsi2 = copy.deepcopy(si)
                nsi2.on_wait = keep
                ins.sync_info = nsi2
                changed = True
            new_list.append(ins)
        if changed:
            blk.instructions = new_list


# ---------------------------------------------------------------------------
# Host-side graph preprocessing: sharding, degree-sort renumbering,
# whole-tile padded-CSR gather indices, masks, per-tile readout indicators.
#
# Global table row for (core c, local degree-sorted row l):
#   chunk = l // (SHARD//N_CHUNK); row = chunk*(N_NODES//N_CHUNK)
#           + (SHARD//N_CHUNK)*c + (l % (SHARD//N_CHUNK))
# so an AllGather of chunk j (concat of all cores' chunk-j shard slices)
# lands records exactly at their global rows.
def _preprocess(edge_index):
    dst = np.asarray(edge_index[0], np.int64)
    src = np.asarray(edge_index[1], np.int64)
    CH_SH = SHARD // N_CHUNK          # local rows per chunk
    CH_GL = N_NODES // N_CHUNK        # global rows per chunk

    perms = []       # per core: local row j -> original local node
    invperms = []    # per core: original local node -> local row
    degs = []
    for c in range(N_CORES):
        sel = (dst >= SHARD * c) & (dst < SHARD * (c + 1))
        dloc = dst[sel] - SHARD * c
        deg = np.bincount(dloc, minlength=SHARD)
        order = np.argsort(-deg, kind="stable")
        inv = np.empty(SHARD, np.int64)
        inv[order] = np.arange(SHARD)
        perms.append(order)
        invperms.append(inv)
        degs.append(deg)

    # renumbered global table row of original node s (chunk-major layout)
    renum = np.empty(N_NODES, np.int64)
    for c in range(N_CORES):
        ell = invperms[c]
        renum[SHARD * c: SHARD * (c + 1)] = (
            (ell // CH_SH) * CH_GL + CH_SH * c + (ell % CH_SH))

    # uniform tile depths across cores
    Dt = np.zeros(TILES, np.int64)
    for c in range(N_CORES):
        sd = degs[c][perms[c]]                      # sorted degrees
        for t in range(TILES):
            Dt[t] = max(Dt[t], sd[128 * t: 128 * (t + 1)].max())
    Dt = np.maximum(Dt, 1)

    # pieces: (tile, k0, kk, first, last, idx_off, mask_off), kk <= KCH
    pieces = []
    ioff = moff = 0
    for t in range(TILES):
        k0 = 0
        while k0 < Dt[t]:
            kk = int(min(KCH, Dt[t] - k0))
            pieces.append((t, k0, kk, k0 == 0, k0 + kk == int(Dt[t]),
                           ioff, moff))
            ioff += 8 * kk
            moff += kk
            k0 += kk
    CI, CM = ioff, moff

    # per-core CSR in renumbered order + idx/mask/indicator buffers
    per_core = []
    for c in range(N_CORES):
        sel = (dst >= SHARD * c) & (dst < SHARD * (c + 1))
        dloc = dst[sel] - SHARD * c
        sglob = src[sel]
        eorder = np.argsort(invperms[c][dloc], kind="stable")
        s_sorted = renum[sglob[eorder]]             # src table rows
        deg_r = degs[c][perms[c]]                   # degree per local row
        starts = np.zeros(SHARD + 1, np.int64)
        starts[1:] = np.cumsum(deg_r)

        idx_buf = np.zeros((128, CI), np.int16)
        mask_buf = np.zeros((128, CM), np.float32)
        for (t, k0, kk, _f, _l, io, mo) in pieces:
            lin = np.zeros(128 * kk, np.int64)
            msk = np.zeros((128, kk), np.float32)
            rows = 128 * t + np.arange(128)
            for j in range(128):
                r = rows[j]
                d = deg_r[r]
                lo, hi = k0, min(d, k0 + kk)
                if hi > lo:
                    e0 = starts[r] + lo
                    kks = np.arange(lo, hi) - k0
                    lin[kks * 128 + j] = s_sorted[e0: e0 + (hi - lo)]
                    msk[j, : hi - lo] = 1.0
            wrapped = lin.astype(np.int16).reshape(-1, 16).T   # [16, 8*kk]
            for g in range(8):
                idx_buf[16 * g: 16 * (g + 1), io: io + 8 * kk] = wrapped
            mask_buf[:, mo: mo + kk] = msk

        ind_buf = np.zeros((128, 16 * TILES), np.float32)
        for t in range(TILES):
            for j in range(128):
                orig = SHARD * c + perms[c][128 * t + j]
                gcol = orig // 512 - 8 * c
                ind_buf[j, 16 * t + gcol] = 1.0
                if orig % 512 == 0:
                    ind_buf[j, 16 * t + 8 + gcol] = 1.0

        per_core.append(dict(idx=idx_buf, mask=mask_buf, ind=ind_buf,
                             perm=perms[c]))

    return pieces, CI, CM, per_core


# ---------------------------------------------------------------------------
def _build(pieces, CI, CM):
    n_dev = 1 if os.environ.get("K_SINGLE") else N_CORES
    nc = bacc.Bacc("TRN2", target_bir_lowering=False, debug=False,
                   num_devices=n_dev, num_swdge_queues=4)
    I = {}
    I["x_perm"] = nc.dram_tensor("x_perm", [SHARD, FT_IN + 1], FP,
                                 kind="ExternalInput")
    I["W1"] = nc.dram_tensor("W1", [FT_IN, HID], FP, kind="ExternalInput")
    I["W2"] = nc.dram_tensor("W2", [HID, HID], FP, kind="ExternalInput")
    I["b1"] = nc.dram_tensor("b1", [1, HID], FP, kind="ExternalInput")
    I["b2"] = nc.dram_tensor("b2", [1, HID], FP, kind="ExternalInput")
    for nm in ("a1s", "a1d", "a2s", "a2d"):
        I[nm] = nc.dram_tensor(nm, [1, HID], FP, kind="ExternalInput")
    I["W_lin"] = nc.dram_tensor("W_lin", [HID + 1, HID + 1], FP,
                                kind="ExternalInput")
    I["lin_scale"] = nc.dram_tensor("lin_scale", [1, 1], FP,
                                    kind="ExternalInput")
    I["idx"] = nc.dram_tensor("idx", [128, CI], mybir.dt.int16,
                              kind="ExternalInput")
    I["mask"] = nc.dram_tensor("mask", [128, CM], FP, kind="ExternalInput")
    I["ind"] = nc.dram_tensor("ind", [128, 16 * TILES], FP,
                              kind="ExternalInput")
    out_sh = nc.dram_tensor("out_shard", [8, HID + 1], FP,
                            kind="ExternalOutput")
    gm_sh = nc.dram_tensor("gm_shard", [8, HID + 1], FP,
                           kind="ExternalOutput")

    REP = int(os.environ.get("K_REPEAT", "1"))
    with tile.TileContext(nc) as tc:
        for _ in range(REP):
            _trace(nc, tc, I, out_sh, gm_sh, pieces)
    nc.compile()
    _split_waits(nc)
    return nc


def _trace(nc, tc, I, out_sh, gm_sh, pieces):
    DMAX = max(kk for (_t, _k0, kk, _f, _l, _io, _mo) in pieces)
    with (
        tc.tile_pool(name="const", bufs=1) as cpool,
        tc.tile_pool(name="io", bufs=2) as iopool,
        tc.tile_pool(name="gat", bufs=6) as gpool,
        tc.tile_pool(name="wg", bufs=2) as wgpool,
        tc.tile_pool(name="vv", bufs=2) as vpool,
        tc.tile_pool(name="sm", bufs=4) as spool,
        tc.tile_pool(name="ps", bufs=1, space="PSUM") as ppool,
        tc.tile_pool(name="pse", bufs=2, space="PSUM") as epool,
        tc.tile_pool(name="ps1", bufs=1, space="PSUM") as ppool1,
        tc.tile_pool(name="psg", bufs=1, space="PSUM") as gmpool,
        tc.tile_pool(name="dram", bufs=1, space="DRAM") as dpool,
    ):
        # ---- constants
        ident = cpool.tile([128, 128], BF)
        masks.make_identity(nc, ident[:])
        ident8 = cpool.tile([8, 8], FP)
        masks.make_identity(nc, ident8[:])
        ident3 = cpool.tile([128, 1, 128], BF)
        masks.make_identity(nc, ident3[:, 0, :])
        ones_row = cpool.tile([1, 128], FP)
        nc.vector.memset(ones_row[:], 1.0)

        idx_all = cpool.tile([128, max(I["idx"].shape[1], 16)],
                             mybir.dt.int16)
        nc.sync.dma_start(idx_all[:, 0:I["idx"].shape[1]], I["idx"].ap())
        mask_all = cpool.tile([128, max(I["mask"].shape[1], 4)], BF)
        nc.gpsimd.dma_start(mask_all[:, 0:I["mask"].shape[1]],
                            I["mask"].ap())
        ind_all = cpool.tile([128, 16 * TILES], FP)
        nc.sync.dma_start(ind_all[:], I["ind"].ap())

        # weights as bf16 lhsT tiles (cast on ACT after f32 load)
        W1f = iopool.tile([128, 2, HID], FP, tag="w1f")
        nc.sync.dma_start(W1f[:, 0, :], I["W1"].ap()[0:128, :])
        nc.sync.dma_start(W1f[:, 1, :], I["W1"].ap()[128:256, :])
        W1sb = cpool.tile([128, 2, HID], BF)
        nc.vector.tensor_copy(W1sb[:].rearrange("p a h -> p (a h)"),
                              W1f[:].rearrange("p a h -> p (a h)"))
        W2f = iopool.tile([128, HID], FP, tag="w2f")
        nc.sync.dma_start(W2f[:], I["W2"].ap())
        W2sb = cpool.tile([128, HID], BF)
        nc.vector.tensor_copy(W2sb[:], W2f[:])
        b1r = cpool.tile([1, HID], FP)
        nc.sync.dma_start(b1r[:], I["b1"].ap())
        b2r = cpool.tile([1, HID], FP)
        nc.sync.dma_start(b2r[:], I["b2"].ap())
        Wlin = cpool.tile([128, HID + 1], FP)
        nc.sync.dma_start(Wlin[:], I["W_lin"].ap()[0:128, :])
        Wlin_l = cpool.tile([1, HID + 1], FP)
        nc.sync.dma_start(Wlin_l[:], I["W_lin"].ap()[128:129, :])
        lsc = cpool.tile([1, 1], FP)
        nc.sync.dma_start(lsc[:], I["lin_scale"].ap())

        # replicated a-vectors via PE outer product with ones
        amats = {}
        for nm in ("a1s", "a1d", "a2s", "a2d"):
            row = cpool.tile([1, HID], FP, tag=f"row_{nm}")
            nc.sync.dma_start(row[:], I[nm].ap())
            ps = ppool.tile([128, HID], FP, tag="tr")
            nc.tensor.matmul(ps[:], ones_row[:], row[:], start=True, stop=True)
            m = cpool.tile([128, HID], FP, tag=f"amat_{nm}")
            nc.vector.tensor_copy(m[:], ps[:])
            amats[nm] = m

        # persistent per-layer state
        sd1 = cpool.tile([128, TILES], FP)
        sd2 = cpool.tile([128, TILES], FP)
        agg_all = cpool.tile([128, TILES, HID], FP, tag="agg_all")
        t2_all = cpool.tile([128, TILES, HID], BF, tag="t2_all")
        agg2_all = cpool.tile([128, TILES, HID], FP, tag="agg2_all")
        h2_all = cpool.tile([128, TILES, HID + 1], FP, tag="h2_all")
        n2_all = cpool.tile([128, TILES], FP, tag="n2_all")
        sc32a = cpool.tile([128, TILES], FP, tag="sc32a")
        sc32b = cpool.tile([128, TILES], FP, tag="sc32b")
        sc32c = cpool.tile([128, TILES], FP, tag="sc32c")
        sc32d = cpool.tile([128, TILES], FP, tag="sc32d")
        xall = cpool.tile([128, TILES, FT_IN + 1], FP, tag="xall")

        tab1_sh = dpool.tile([SHARD, REC], BF)
        tab1 = dpool.tile([N_NODES, REC], BF)
        tab2_sh = dpool.tile([SHARD, REC], BF)
        tab2 = dpool.tile([N_NODES, REC], BF)

        CH_SH = SHARD // N_CHUNK
        CH_GL = N_NODES // N_CHUNK

        def ag_chunk(tab_sh, tab, j):
            if os.environ.get("K_SINGLE"):
                nc.sync.dma_start(
                    tab[CH_GL * j: CH_GL * j + CH_SH, :],
                    tab_sh[CH_SH * j: CH_SH * (j + 1), :])
            else:
                nc.gpsimd.collective_compute(
                    "AllGather", ALU.bypass,
                    replica_groups=[list(range(N_CORES))],
                    ins=[tab_sh[CH_SH * j: CH_SH * (j + 1), :].opt()],
                    outs=[tab[CH_GL * j: CH_GL * (j + 1), :].opt()])

        def node_phase(t, tanT_parts, brow, ams, amd, sd_t, tab_shard):
            """z = tan @ W + b into PSUM; record row + s_src/s_dst."""
            z_ps = ppool.tile([128, HID], FP, tag="z")
            for i, (tT, Wp) in enumerate(tanT_parts):
                nc.tensor.matmul(z_ps[:], tT[:], Wp, start=(i == 0),
                                 stop=False)
            nc.tensor.matmul(z_ps[:], ones_row[:], brow[:], start=False,
                             stop=True)
            stg = iopool.tile([128, REC], BF, tag="stg")
            nc.scalar.copy(stg[:, 0:HID], z_ps[:])
            scr = vpool.tile([128, HID], FP, tag="scr")
            nc.vector.tensor_tensor(scr[:], z_ps[:], ams[:], ALU.mult)
            ssf = spool.tile([128, 1], FP, tag="ssf")
            nc.vector.tensor_reduce(ssf[:], scr[:],
                                    axis=mybir.AxisListType.X, op=ALU.add)
            nc.vector.tensor_copy(stg[:, HID:HID + 1], ssf[:])
            scr2 = vpool.tile([128, HID], FP, tag="scr2")
            nc.vector.tensor_tensor(scr2[:], z_ps[:], amd[:], ALU.mult)
            nc.vector.tensor_reduce(sd_t[:, t:t + 1], scr2[:],
                                    axis=mybir.AxisListType.X, op=ALU.add)
            nc.sync.dma_start(tab_shard[128 * t:128 * (t + 1), :], stg[:])

        # ============ phase A1: logmap (batched ACT) + conv1 node part ====
        nc.sync.dma_start(
            xall[:], I["x_perm"].ap().rearrange("(t p) f -> p t f", p=128))
        for t in range(TILES):
            scr = vpool.tile([128, FT_IN], FP, tag="scrA")
            nc.vector.tensor_tensor(scr[:], xall[:, t, 1:FT_IN + 1],
                                    xall[:, t, 1:FT_IN + 1], ALU.mult)
            nc.vector.tensor_reduce(n2_all[:, t:t + 1], scr[:],
                                    axis=mybir.AxisListType.X, op=ALU.add)
        nn_a = sc32a
        nc.scalar.sqrt(nn_a[:], n2_all[:])                      # 1 table load
        npx = sc32b
        nc.vector.tensor_tensor(npx[:], nn_a[:], xall[:, :, 0], ALU.add)
        lt = sc32c
        nc.scalar.activation(lt[:], npx[:], AF.Ln)              # 1 table load
        rn = sc32d
        nc.vector.reciprocal(rn[:], nn_a[:])
        cf_a = sc32b                                            # reuse
        nc.vector.tensor_tensor(cf_a[:], lt[:], rn[:], ALU.mult)

        for j in range(N_CHUNK):
            for t in range(CTILES * j, CTILES * (j + 1)):
                tan = iopool.tile([128, FT_IN], BF, tag="tan")
                nc.scalar.mul(tan[:], xall[:, t, 1:FT_IN + 1],
                              cf_a[:, t:t + 1])
                parts = []
                for h in range(2):
                    tps = ppool.tile([128, 128], BF, tag="tr")
                    nc.tensor.transpose(tps[:],
                                        tan[:, 128 * h:128 * (h + 1)],
                                        ident[:])
                    tsb = iopool.tile([128, 128], BF, tag=f"tT{h}")
                    nc.vector.tensor_copy(tsb[:], tps[:])
                    parts.append((tsb, W1sb[:, h, :]))
                node_phase(t, parts, b1r, amats["a1s"], amats["a1d"],
                           sd1, tab1_sh)
            ag_chunk(tab1_sh, tab1, j)

        phases = os.environ.get("K_PHASES", "full")
        if phases == "a1":
            z0 = cpool.tile([8, HID + 1], FP, tag="zero")
            nc.vector.memset(z0[:], 0.0)
            nc.sync.dma_start(out_sh.ap(), z0[:])
            nc.sync.dma_start(gm_sh.ap(), z0[:])
            return

        # ======= edge phase (pieces of <=KCH slots; 4 SWDGE queues) ====
        qctr = [0]

        def edge_piece(tab, piece, sd_t, agg_out_t, state):
            (t, k0, kk, first, last, io, mo) = piece
            G = gpool.tile([128, DMAX, REC], BF, tag="G")
            nc.gpsimd.dma_gather(
                out_ap=G[:, 0:kk, :], in_ap=tab[:, :],
                idxs_ap=idx_all[:, io:io + 8 * kk],
                num_idxs=128 * kk, num_idxs_reg=128 * kk, elem_size=REC,
                single_packet=False, queue_num=qctr[0] % 4)
            qctr[0] += 1
            # attention weights [128, kk]
            w = spool.tile([128, DMAX], FP, tag="w")
            nc.vector.tensor_tensor(w[:, 0:kk], G[:, 0:kk, HID],
                                    sd_t[:, t:t + 1].broadcast_to([128, kk]),
                                    ALU.add)
            nc.vector.scalar_tensor_tensor(
                w[:, 0:kk], w[:, 0:kk], 0.2, w[:, 0:kk], ALU.mult, ALU.max)
            nc.scalar.activation(w[:, 0:kk], w[:, 0:kk], AF.Exp)
            wm = spool.tile([128, DMAX, 1], BF, tag="wm")
            nc.vector.tensor_tensor(wm[:, 0:kk, 0], w[:, 0:kk],
                                    mask_all[:, mo:mo + kk], ALU.mult)
            if first:
                dn = spool.tile([128, 1], FP, tag="dn")
                state["dn"] = dn
                nc.vector.tensor_reduce(dn[:], wm[:, 0:kk, 0],
                                        axis=mybir.AxisListType.X, op=ALU.add)
            else:
                dnp = spool.tile([128, 1], FP, tag="dnp")
                nc.vector.tensor_reduce(dnp[:], wm[:, 0:kk, 0],
                                        axis=mybir.AxisListType.X, op=ALU.add)
                nc.vector.tensor_tensor(state["dn"][:], state["dn"][:],
                                        dnp[:], ALU.add)
            # diag bank: diag(wm_k) for each slot, then PE MAC into PSUM
            DB = wgpool.tile([128, DMAX, HID], BF, tag="DB")
            nc.vector.tensor_tensor(
                DB[:, 0:kk, :],
                wm[:, 0:kk, :].broadcast_to([128, kk, HID]),
                ident3[:, 0:1, :].broadcast_to([128, kk, HID]), ALU.mult)
            if first:
                agg_ps = epool.tile([128, HID], FP, tag="aggps")
                state["agg_ps"] = agg_ps
            agg_ps = state["agg_ps"]
            for k in range(kk):
                nc.tensor.matmul(agg_ps[:], DB[:, k, :], G[:, k, 0:HID],
                                 start=(first and k == 0),
                                 stop=(last and k == kk - 1))
            if last:
                dn2 = spool.tile([128, 1], FP, tag="dn2")
                nc.vector.tensor_scalar_max(dn2[:], state["dn"][:], EPS)
                rcp = spool.tile([128, 1], FP, tag="rcp")
                nc.vector.reciprocal(rcp[:], dn2[:])
                nc.vector.tensor_tensor(
                    agg_out_t, agg_ps[:],
                    rcp[:, 0:1].broadcast_to([128, HID]), ALU.mult)

        def edge_tiles(tab, sd_t, agg_all_t, t_lo, t_hi):
            state = {}
            for piece in pieces:
                t = piece[0]
                if t_lo <= t < t_hi:
                    edge_piece(tab, piece, sd_t, agg_all_t[:, t, :], state)

        # ---- layer 1 edge + layer 2 node, interleaved per chunk
        for j in range(N_CHUNK):
            edge_tiles(tab1, sd1, agg_all, CTILES * j, CTILES * (j + 1))
            # batched gelu for this chunk (single table load per chunk)
            nc.scalar.activation(
                t2_all[:, CTILES * j:CTILES * (j + 1), :].rearrange(
                    "p t f -> p (t f)"),
                agg_all[:, CTILES * j:CTILES * (j + 1), :].rearrange(
                    "p t f -> p (t f)"),
                AF.Gelu_apprx_tanh)
            for t in range(CTILES * j, CTILES * (j + 1)):
                tps = ppool.tile([128, 128], BF, tag="tr")
                nc.tensor.transpose(tps[:], t2_all[:, t, :], ident[:])
                tsb = iopool.tile([128, 128], BF, tag="t2T")
                nc.vector.tensor_copy(tsb[:], tps[:])
                node_phase(t, [(tsb, W2sb[:])], b2r, amats["a2s"],
                           amats["a2d"], sd2, tab2_sh)
            ag_chunk(tab2_sh, tab2, j)

        if phases == "l1":
            z0 = cpool.tile([8, HID + 1], FP, tag="zero")
            nc.vector.memset(z0[:], 0.0)
            nc.sync.dma_start(out_sh.ap(), z0[:])
            nc.sync.dma_start(gm_sh.ap(), z0[:])
            return

        # ---- layer 2 edge
        edge_tiles(tab2, sd2, agg2_all, 0, TILES)
        for t in range(TILES):
            scr = vpool.tile([128, HID], FP, tag="e_scr")
            nc.vector.tensor_tensor(scr[:], agg2_all[:, t, :],
                                    agg2_all[:, t, :], ALU.mult)
            nc.vector.tensor_reduce(n2_all[:, t:t + 1], scr[:],
                                    axis=mybir.AxisListType.X, op=ALU.add)
        # batched expmap scalars: nn, sinh(n)/n
        nn_e = sc32a
        nc.scalar.sqrt(nn_e[:], n2_all[:])                      # table load
        ep = sc32b
        nc.scalar.activation(ep[:], nn_e[:], AF.Exp)            # table load
        em = sc32c
        nc.scalar.activation(em[:], nn_e[:], AF.Exp, scale=-1.0)
        sh = sc32b                                              # reuse ep slot
        nc.vector.tensor_tensor(sh[:], ep[:], em[:], ALU.subtract)
        nm = sc32c
        nc.vector.tensor_scalar_max(nm[:], nn_e[:], EPS)
        rn_e = sc32d
        nc.vector.reciprocal(rn_e[:], nm[:])
        cf_e = sc32b
        nc.vector.tensor_tensor(cf_e[:], sh[:], rn_e[:], ALU.mult)
        nc.vector.tensor_scalar_mul(cf_e[:], cf_e[:], 0.5)
        hn2_all = sc32c
        for t in range(TILES):
            nc.scalar.mul(h2_all[:, t, 1:HID + 1], agg2_all[:, t, :],
                          cf_e[:, t:t + 1])
            scr = vpool.tile([128, HID], FP, tag="e_scr2")
            nc.vector.tensor_tensor(scr[:], h2_all[:, t, 1:HID + 1],
                                    h2_all[:, t, 1:HID + 1], ALU.mult)
            nc.vector.tensor_reduce(hn2_all[:, t:t + 1], scr[:],
                                    axis=mybir.AxisListType.X, op=ALU.add)
        # h0 = sqrt(1 + |hs|^2), strided write into h2_all[:, :, 0]
        nc.scalar.activation(h2_all[:, :, 0], hn2_all[:], AF.Sqrt, bias=1.0)

        gm_ps = gmpool.tile([8, HID + 1], FP, tag="gmA")
        g_ps = gmpool.tile([8, HID + 1], FP, tag="gmB")
        for t in range(TILES):
            nc.tensor.matmul(gm_ps[:], ind_all[:, 16 * t:16 * t + 8],
                             h2_all[:, t, :], start=(t == 0),
                             stop=(t == TILES - 1))
            nc.tensor.matmul(g_ps[:], ind_all[:, 16 * t + 8:16 * (t + 1)],
                             h2_all[:, t, :], start=(t == 0),
                             stop=(t == TILES - 1))

        # ================= readout =================
        g = cpool.tile([8, HID + 1], FP, tag="f_g")
        nc.vector.tensor_copy(g[:], g_ps[:])
        ave = cpool.tile([8, HID + 1], FP)
        nc.scalar.mul(ave[:], gm_ps[:], 1.0 / 512.0)
        q = cpool.tile([8, 1], FP, tag="f_q")
        scr = vpool.tile([8, HID], FP, tag="f_scr")
        nc.vector.tensor_tensor(scr[:], ave[:, 1:HID + 1],
                                ave[:, 1:HID + 1], ALU.mult)
        nc.vector.tensor_reduce(q[:], scr[:],
                                axis=mybir.AxisListType.X, op=ALU.add)
        t0s = cpool.tile([8, 1], FP, tag="f_t0s")
        nc.vector.tensor_tensor(t0s[:], ave[:, 0:1], ave[:, 0:1], ALU.mult)
        dif = cpool.tile([8, 1], FP, tag="f_dif")
        nc.vector.tensor_tensor(dif[:], t0s[:], q[:], ALU.subtract)
        nc.vector.tensor_scalar_max(dif[:], dif[:], 1e-8)
        dsq = cpool.tile([8, 1], FP, tag="f_dsq")
        nc.scalar.sqrt(dsq[:], dif[:])
        rr = cpool.tile([8, 1], FP, tag="f_rr")
        nc.vector.reciprocal(rr[:], dsq[:])
        gm = cpool.tile([8, HID + 1], FP, tag="f_gm")
        nc.scalar.mul(gm[:], ave[:], rr[:, 0:1])
        nc.sync.dma_start(gm_sh.ap(), gm[:])

        # y = g @ W_lin
        gT_ps = ppool.tile([128, 8], FP, tag="tr")
        nc.tensor.transpose(gT_ps[:], g[:, 0:128], ident8[:])
        gT = cpool.tile([128, 8], FP, tag="f_gT")
        nc.vector.tensor_copy(gT[:], gT_ps[:])
        gl_ps = ppool1.tile([1, 8], FP, tag="tr2")
        nc.tensor.transpose(gl_ps[:], g[:, 128:129], ident8[:])
        gl = cpool.tile([1, 8], FP, tag="f_gl")
        nc.vector.tensor_copy(gl[:], gl_ps[:])
        y_ps = ppool1.tile([8, HID + 1], FP, tag="y")
        nc.tensor.matmul(y_ps[:], gT[:], Wlin[:], start=True, stop=False)
        nc.tensor.matmul(y_ps[:], gl[:], Wlin_l[:], start=False, stop=True)
        y = cpool.tile([8, HID + 1], FP, tag="f_y")
        nc.vector.tensor_copy(y[:], y_ps[:])

        ls_ps = ppool1.tile([8, 1], FP, tag="tr2")
        ones8 = cpool.tile([1, 8], FP, tag="f_ones8")
        nc.vector.memset(ones8[:], 1.0)
        nc.tensor.matmul(ls_ps[:], ones8[:], lsc[:], start=True, stop=True)
        lsb = cpool.tile([8, 1], FP, tag="f_lsb")
        nc.vector.tensor_copy(lsb[:], ls_ps[:])

        sig = cpool.tile([8, 1], FP, tag="f_sig")
        nc.scalar.activation(sig[:], y[:, 0:1], AF.Sigmoid)
        tme = cpool.tile([8, 1], FP, tag="f_tme")
        nc.vector.tensor_tensor(tme[:], sig[:], lsb[:], ALU.mult)
        nc.vector.tensor_scalar_add(tme[:], tme[:], 1.1)
        s2 = cpool.tile([8, 1], FP, tag="f_s2")
        scr2 = vpool.tile([8, HID], FP, tag="f_scr2")
        nc.vector.tensor_tensor(scr2[:], y[:, 1:HID + 1],
                                y[:, 1:HID + 1], ALU.mult)
        nc.vector.tensor_reduce(s2[:], scr2[:],
                                axis=mybir.AxisListType.X, op=ALU.add)
        nc.vector.tensor_scalar_max(s2[:], s2[:], 1e-8)
        rs2 = cpool.tile([8, 1], FP, tag="f_rs2")
        nc.vector.reciprocal(rs2[:], s2[:])
        tm1 = cpool.tile([8, 1], FP, tag="f_tm1")
        nc.vector.scalar_tensor_tensor(tm1[:], tme[:], 1.0, tme[:],
                                       ALU.mult, ALU.mult)
        nc.vector.tensor_scalar_add(tm1[:], tm1[:], -1.0)
        fac2 = cpool.tile([8, 1], FP, tag="f_fac2")
        nc.vector.tensor_tensor(fac2[:], tm1[:], rs2[:], ALU.mult)
        fac = cpool.tile([8, 1], FP, tag="f_fac")
        nc.scalar.sqrt(fac[:], fac2[:])
        outt = cpool.tile([8, HID + 1], FP, tag="f_out")
        nc.vector.tensor_copy(outt[:, 0:1], tme[:])
        nc.scalar.mul(outt[:, 1:HID + 1], y[:, 1:HID + 1], fac[:, 0:1])
        nc.sync.dma_start(out_sh.ap(), outt[:])


_CACHE = {}


def _get_compiled(edge_index):
    key = hash(np.asarray(edge_index).tobytes())
    if key not in _CACHE:
        pieces, CI, CM, per_core = _preprocess(edge_index)
        nc = _build(pieces, CI, CM)
        _CACHE[key] = (nc, per_core)
    return _CACHE[key]


def _make_in_maps(x, per_core, W1, b1, a1_src, a1_dst, W2, b2, a2_src,
                  a2_dst, W_lin, lin_scale):
    in_maps = []
    for c in range(N_CORES):
        pc = per_core[c]
        xp = x[SHARD * c + pc["perm"], :]
        in_maps.append(dict(
            x_perm=np.ascontiguousarray(xp),
            W1=np.asarray(W1, np.float32),
            W2=np.asarray(W2, np.float32),
            b1=np.asarray(b1, np.float32).reshape(1, HID),
            b2=np.asarray(b2, np.float32).reshape(1, HID),
            a1s=np.asarray(a1_src, np.float32).reshape(1, HID),
            a1d=np.asarray(a1_dst, np.float32).reshape(1, HID),
            a2s=np.asarray(a2_src, np.float32).reshape(1, HID),
            a2d=np.asarray(a2_dst, np.float32).reshape(1, HID),
            W_lin=np.asarray(W_lin, np.float32),
            lin_scale=np.asarray(lin_scale, np.float32).reshape(1, 1),
            idx=pc["idx"], mask=pc["mask"], ind=pc["ind"],
        ))
    return in_maps


def kernel(x, edge_index, batch_size, W1, b1, a1_src, a1_dst,
           W2, b2, a2_src, a2_dst, W_lin, lin_scale, _trace=False):
    x = np.asarray(x, np.float32)
    assert int(batch_size) == BATCH
    nc, per_core = _get_compiled(edge_index)
    in_maps = _make_in_maps(x, per_core, W1, b1, a1_src, a1_dst, W2, b2,
                            a2_src, a2_dst, W_lin, lin_scale)
    res = run_bass_kernel_spmd(nc, in_maps, core_ids=list(range(N_CORES)),
                               trace=_trace)
    out = np.concatenate([res.results[c]["out_shard"]
                          for c in range(N_CORES)], 0)
    gm = np.concatenate([res.results[c]["gm_shard"]
                         for c in range(N_CORES)], 0)
    if _trace:
        kernel.last_exec_time_ns = res.exec_time_ns
        kernel.last_results = res
    return (out, gm)


kernel.last_exec_time_ns = None
